# revision 24
# baseline (speedup 1.0000x reference)
"""Trainium2 Bass kernel for cached multi-head self-attention decode step.

Problem (hardcoded):
  B=16, T=8, C=1024, n_head=16, head_dim=64, Lcache=4096, layer index 1.
  reference:
    q = x@Wq.T + bq ; key = x@Wk.T ; value = x@Wv.T + bv
    K = concat(kv_cache[:,1,0], key) ; V = concat(kv_cache[:,1,1], value)
    out = softmax((q*s)(K*s)^T) @ V @ Wo.T + bo      (s = hd**-0.25)
    returns (out, key, value)

Sharding: data-parallel over batch. 8 cores x 2 batches each. No collectives.

v4 design:
  - fp8 DoubleRow matmuls (contract 256/instr, measured ~225ns warm at
    N=512 - 2x bf16 per contract) for scores, S@V, q-proj, out-proj.
  - all transposes via matmul with identity rhs.
  - ONE ordered sync-queue DMA stream; PE program order aligned with DMA
    completion order so the PE never head-of-line blocks:
      x,Wq | KT0 | V0s0 | K1w01 | Wk | V0s1 | K1w23 | V0s2 | K1w45 |
      V0s3 | K1w67 | Wv | V1 | Wo
    PE: warmup, q, scores0+T0, sv0(c0-3), s1w01+T1, kproj+kT+newkey0,
      sv0(c4-7), s1w23, sv0(c8-11), s1w45, sv0(c12-15), s1w67, vproj,
      svfinal0, On0, newkey1, gather0, sv1, svfinal1, On1, gather1+outproj.
  - PE kept dense so the HAM clock gate stays at 8/8 (cold MMs are 2x).
"""

import sys
import types

import numpy as np
import ml_dtypes

# ---- hardcoded problem geometry ----
B, T, C = 16, 8, 1024
H, HD = 16, 64
L = 4096            # cached length
LT = L + T          # total keys
NCORES = 8
BPC = B // NCORES   # batches per core = 2
M = BPC * T         # queries per core = 16
P = 128
CH = C // P         # 8 c-chunks
NW = L // 512       # 8 score windows of 512
NV = 8              # V l-chunks (128 rows) per DMA tile (1MB transfers)
NDR = L // 256      # 16 DoubleRow l-pair chunks per batch
NWARM = 26          # PE warmup matmuls of N=512 (HAM un-throttle + stay busy
#                     until the first weights land)
SCALE = float(HD) ** -0.5  # folded into Wq/bq on host

# softmax logit shift: exp(s + ESHIFT); cancels in normalization, keeps the
# fp8 S@V weights well inside e4m3 range.
ESHIFT = -2.0

_CACHE = {}


def _ensure_ntff_hook():
    """run_bass_kernel_spmd(trace=True) under axon needs antenv.axon_hooks;
    shim it from the boot module if the image's antenv lacks it."""
    try:
        import antenv.axon_hooks  # noqa: F401
        return
    except ImportError:
        pass
    try:
        import trn_agent_boot.trn_boot as tb
        hook = tb._ntff_profile_via_ctypes("/opt/axon/libaxon_pjrt.so")
    except Exception:
        hook = None
    mod = types.ModuleType("antenv.axon_hooks")
    mod.get_axon_ntff_profile_hook = lambda: hook
    mod.set_axon_ntff_profile_hook = lambda h: None
    sys.modules["antenv.axon_hooks"] = mod


def _build():
    import concourse.bacc as bacc
    import concourse.mybir as mybir
    import concourse.tile as tile
    from concourse.masks import make_identity

    f32 = mybir.dt.float32
    bf16 = mybir.dt.bfloat16
    fp8 = mybir.dt.float8e4
    DR = mybir.MatmulPerfMode.DoubleRow

    nc = bacc.Bacc(None, target_bir_lowering=False)

    # ---- dram I/O (all host-repacked for contiguous loads) ----
    xT8 = nc.dram_tensor("xT8", [P, CH, M], fp8, kind="ExternalInput")
    xTb = nc.dram_tensor("xTb", [P, CH, M], bf16, kind="ExternalInput")
    Wq8 = nc.dram_tensor("Wq8", [P, CH * C], fp8, kind="ExternalInput")
    Wo8 = nc.dram_tensor("Wo8", [P, CH * C], fp8, kind="ExternalInput")
    Wkvd = nc.dram_tensor("Wkvd", [2, P, CH * C], bf16, kind="ExternalInput")
    KT = nc.dram_tensor("KT", [BPC, NW, P, CH * 512], fp8, kind="ExternalInput")
    Vd = nc.dram_tensor("Vd", [BPC, L // (P * NV), P, NV * C], fp8,
                        kind="ExternalInput")
    bqs = nc.dram_tensor("bqs", [P, CH], f32, kind="ExternalInput")
    bvb = nc.dram_tensor("bvb", [M, C], f32, kind="ExternalInput")
    bob = nc.dram_tensor("bob", [M, C], f32, kind="ExternalInput")
    out_d = nc.dram_tensor("out", [M, C], f32, kind="ExternalOutput")
    key_d = nc.dram_tensor("key", [M, C], f32, kind="ExternalOutput")
    val_d = nc.dram_tensor("value", [M, C], f32, kind="ExternalOutput")

    AF = mybir.ActivationFunctionType
    AX = mybir.AxisListType
    OP = mybir.AluOpType

    with tile.TileContext(nc) as tc:
        with (
            tc.tile_pool(name="const", bufs=1) as cpool,
            tc.tile_pool(name="kt", bufs=15) as ktpool,
            tc.tile_pool(name="v", bufs=6) as vpool,
            tc.tile_pool(name="nat", bufs=2) as natpool,
            tc.tile_pool(name="wchunk", bufs=2 * NW + 2) as wtpool,
            tc.tile_pool(name="big", bufs=1) as big,
            tc.tile_pool(name="ps", bufs=1, space="PSUM") as pp,
        ):
            # ---------------- DMA issue order == consumption order --------
            xT8_sb = cpool.tile([P, CH, M], fp8, tag="xT8", name="xT8")
            nc.sync.dma_start(xT8_sb[:], xT8[:])
            xTb_sb = cpool.tile([P, CH, M], bf16, tag="xTb", name="xTb")
            nc.sync.dma_start(xTb_sb[:], xTb[:])
            wq_sb = cpool.tile([P, CH, C], fp8, tag="wq", name="wq")
            nc.sync.dma_start(wq_sb[:], Wq8[:])
            kts = {}

            def kt_dma(b, w):
                kts[(b, w)] = ktpool.tile([P, CH, 512], fp8, tag="kt",
                                          name="kt")
                nc.sync.dma_start(kts[(b, w)][:], KT[b, w])

            vts = {}

            def v_dma(b, s):
                vts[(b, s)] = vpool.tile([P, NV, C], fp8, tag="v", name="v")
                nc.sync.dma_start(vts[(b, s)][:], Vd[b, s])

            wkv_sb = cpool.tile([P, 2, CH, C], bf16, tag="wkv", name="wkv")

            for w in range(NW):
                kt_dma(0, w)
            v_dma(0, 0)
            kt_dma(1, 0); kt_dma(1, 1)
            nc.sync.dma_start(wkv_sb[:, 0], Wkvd[0])   # Wk
            v_dma(0, 1)
            kt_dma(1, 2); kt_dma(1, 3)
            v_dma(0, 2)
            kt_dma(1, 4); kt_dma(1, 5)
            v_dma(0, 3)
            kt_dma(1, 6); kt_dma(1, 7)
            nc.sync.dma_start(wkv_sb[:, 1], Wkvd[1])   # Wv
            for s in range(4):
                v_dma(1, s)
            wo_sb = cpool.tile([P, CH, C], fp8, tag="wo", name="wo")
            nc.sync.dma_start(wo_sb[:], Wo8[:])

            # ---- constants / memsets (gpsimd memsets BEFORE its slow
            # SWDGE dma emissions so the PE warmup input is ready early) ----
            warm_in = cpool.tile([P, 512], bf16, tag="warm_in", name="warm_in")
            nc.gpsimd.memset(warm_in[:], 0.25)
            eshift = cpool.tile([P, 1], f32, tag="eshift", name="eshift")
            nc.gpsimd.memset(eshift[:], ESHIFT)
            Qb = {}
            for b in range(BPC):
                Qb[b] = big.tile([P, CH, P], fp8, tag=f"Qbig{b}",
                                 name=f"Qbig{b}")
                nc.gpsimd.memset(Qb[b][:], 0.0)
            vpad = big.tile([P, C], fp8, tag="vpad", name="vpad")
            nc.gpsimd.memset(vpad[:], 0.0)
            wn_pad, wt32 = {}, {}
            for b in range(BPC):
                wn_pad[b] = big.tile([P, M], fp8, tag=f"wn_pad{b}",
                                     name=f"wn_pad{b}")
                nc.gpsimd.memset(wn_pad[b][:], 0.0)
                wt32[b] = big.tile([P, P], fp8, tag=f"wt32_{b}",
                                   name=f"wt32_{b}")
                nc.gpsimd.memset(wt32[b][:], 0.0)
            bqs_sb = cpool.tile([P, CH], f32, tag="bqs", name="bqs")
            nc.gpsimd.dma_start(bqs_sb[:], bqs[:])
            bvb_sb = cpool.tile([M, C], bf16, tag="bvb", name="bvb")
            nc.gpsimd.dma_start(bvb_sb[:], bvb[:])
            bob_sb = cpool.tile([M, C], bf16, tag="bob", name="bob")
            nc.gpsimd.dma_start(bob_sb[:], bob[:])

            ident = cpool.tile([P, P], f32, tag="ident", name="ident")
            make_identity(nc, ident)
            ident_b = cpool.tile([P, P], bf16, tag="ident_b", name="ident_b")
            nc.vector.tensor_copy(out=ident_b[:], in_=ident[:])
            ident_8 = cpool.tile([P, P], fp8, tag="ident_8", name="ident_8")
            nc.vector.tensor_copy(out=ident_8[:], in_=ident[:])

            # ---- PE warmup / filler: keep the HAM clock gate at 8/8 (cold
            # matmuls run at 1.2 vs 2.4 GHz). Each call allocates from the
            # t-tag ring so WAR deps are tracked.
            _wk = [0]

            def warm_fill(n, nfree=64):
                wm = pp.tile([P, 512], f32, tag=f"t{_wk[0] % 2}", name="warm")
                _wk[0] += 1
                for _ in range(n):
                    nc.tensor.matmul(wm[:, 0:nfree], warm_in[:, 0:P],
                                     warm_in[:, 0:nfree],
                                     start=True, stop=True)

            warm_fill(NWARM, nfree=512)

            # ---------------- per-batch state ------------------------------
            W_s, sums, rsum, ops_b, On = {}, {}, {}, {}, {}
            for b in range(BPC):
                W_s[b] = big.tile([P, LT], fp8, tag=f"W{b}", name=f"W{b}")
                sums[b] = big.tile([P, NW + 1], f32, tag=f"sums{b}",
                                   name=f"sums{b}")
                On[b] = big.tile([P, C], bf16, tag=f"On{b}", name=f"On{b}")
            wts = {0: {}, 1: {}}  # (b -> chunk -> wt tile)

            # ---------------- Phase A: q projection (fp8 DR) --------------
            q_bf = big.tile([M, C], bf16, tag="q_bf", name="q_bf")
            for j in range(2):
                qps = pp.tile([M, 512], f32, tag=f"s{j}", name=f"qps{j}")
                for ci in range(0, CH, 2):
                    nc.tensor.matmul(
                        qps[:], xT8_sb[:, ci:ci + 2, :],
                        wq_sb[:, ci:ci + 2, j * 512:(j + 1) * 512],
                        start=(ci == 0), stop=(ci == CH - 2), perf_mode=DR,
                    )
                nc.scalar.copy(q_bf[:, j * 512:(j + 1) * 512], qps[:])
            for co in range(CH):
                tpq = pp.tile([P, P], f32, tag=f"t{co % 2}", name="tpq")
                nc.tensor.matmul(
                    tpq[:, 0:M], q_bf[:, co * P:(co + 1) * P],
                    ident_b[0:M, 0:M], start=True, stop=True,
                )
                for b in range(BPC):
                    for j in range(2):
                        rows = slice(64 * j, 64 * (j + 1))
                        nc.scalar.activation(
                            Qb[b][rows, co, 16 * co + 8 * j:16 * co + 8 * j + 8],
                            tpq[rows, b * T:b * T + T],
                            AF.Identity, bias=bqs_sb[rows, co:co + 1],
                        )

            def w_transpose(b, t):
                """wt pair tile for DR l-chunk t of batch b (2 matmul-T)."""
                wt = wtpool.tile([P, 2, P], fp8, tag="wt", name="wt")
                for i in range(2):
                    tpw = pp.tile([P, P], f32, tag=f"t{(2 * t + i) % 2}",
                                  name="tpw")
                    nc.tensor.matmul(
                        tpw[:], W_s[b][:, (2 * t + i) * P:(2 * t + i + 1) * P],
                        ident_8[:], start=True, stop=True,
                    )
                    nc.vector.tensor_copy(out=wt[:, i, :], in_=tpw[:])
                wts[b][t] = wt

            def scores_window(b, lw, transpose=True):
                kt = kts[(b, lw)]
                sp = pp.tile([P, 512], f32, tag=f"s{lw % 2}", name="sp")
                for ci in range(0, CH, 2):
                    nc.tensor.matmul(
                        sp[:], Qb[b][:, ci:ci + 2, :], kt[:, ci:ci + 2, :],
                        start=(ci == 0), stop=(ci == CH - 2), perf_mode=DR,
                    )
                nc.scalar.activation(
                    W_s[b][:, lw * 512:(lw + 1) * 512], sp[:], AF.Exp,
                    bias=eshift[:, 0:1],
                    accum_out=sums[b][:, lw:lw + 1],
                )
                if transpose and lw > 0:
                    w_transpose(b, 2 * (lw - 1))
                    w_transpose(b, 2 * (lw - 1) + 1)

            def scores_tail(b):
                w_transpose(b, 2 * (NW - 1))
                w_transpose(b, 2 * (NW - 1) + 1)

            def sv_chunks(b, t0, t1):
                """pure DR S@V over l-pair chunks [t0, t1)."""
                if t0 == 0:
                    ops_b[b] = [pp.tile([P, 512], f32, tag=f"o{2 * b + j}",
                                        name=f"sv{b}{j}") for j in range(2)]
                ops = ops_b[b]
                for t_ in range(t0, t1):
                    vt = vts[(b, t_ // 4)]
                    tt = t_ % 4
                    for j in range(2):
                        nc.tensor.matmul(
                            ops[j][:], wts[b][t_][:],
                            vt[:, 2 * tt:2 * tt + 2, j * 512:(j + 1) * 512],
                            start=(t_ == 0), stop=False, perf_mode=DR,
                        )
                    del wts[b][t_]

            def scores_newkey_a(b):
                # new-key scores + normalization sums + wn_pad staging; the
                # DVE tail here runs under whatever PE work follows.
                spn = pp.tile([P, 512], f32, tag=f"s{b % 2}", name="spn")
                for ci in range(0, CH, 2):
                    nc.tensor.matmul(
                        spn[:, 0:T], Qb[b][:, ci:ci + 2, :],
                        kT[:, ci:ci + 2, b * T:(b + 1) * T],
                        start=(ci == 0), stop=(ci == CH - 2), perf_mode=DR,
                    )
                nc.scalar.activation(
                    W_s[b][:, L:LT], spn[:, 0:T], AF.Exp,
                    bias=eshift[:, 0:1], accum_out=sums[b][:, NW:NW + 1],
                )
                rs = big.tile([P, 1], f32, tag=f"rs{b}", name=f"rs{b}")
                nc.vector.tensor_reduce(out=rs[:], in_=sums[b][:],
                                        axis=AX.X, op=OP.add)
                rsum[b] = big.tile([P, 1], f32, tag=f"rsum{b}", name=f"rsum{b}")
                nc.vector.reciprocal(rsum[b][:], rs[:])
                nc.vector.tensor_copy(out=wn_pad[b][:, b * T:(b + 1) * T],
                                      in_=W_s[b][:, L:LT])

            def scores_newkey_b(b):
                tpn = pp.tile([P, P], f32, tag=f"t{b % 2}", name="tpn")
                nc.tensor.matmul(tpn[0:M, :], wn_pad[b][:], ident_8[:],
                                 start=True, stop=True)
                nc.vector.tensor_copy(out=wt32[b][0:M, :], in_=tpn[0:M, :])

            def sv_final(b):
                ops = ops_b[b]
                for j in range(2):
                    nc.tensor.matmul(
                        ops[j][:], wt32[b][:], vpad[:, j * 512:(j + 1) * 512],
                        start=False, stop=True,
                    )
                    nc.scalar.activation(
                        On[b][:, j * 512:(j + 1) * 512], ops[j][:], AF.Copy,
                        scale=rsum[b][:],
                    )

            wvT = big.tile([P, CH, M], fp8, tag="wvT", name="wvT")

            def gather_ci(b, ci):
                tp = pp.tile([P, P], f32, tag=f"t{ci % 2}", name="tpg")
                nc.tensor.matmul(tp[:], On[b][:, ci * P:(ci + 1) * P],
                                 ident_b[:], start=True, stop=True)
                nc.vector.tensor_copy(
                    out=wvT[0:64, ci, b * T:(b + 1) * T],
                    in_=tp[0:64, 16 * ci:16 * ci + 8])
                nc.vector.tensor_copy(
                    out=wvT[64:P, ci, b * T:(b + 1) * T],
                    in_=tp[64:P, 16 * ci + 8:16 * ci + 16])

            # ---- k/v natural projection pieces (bf16) ----
            k_nat = natpool.tile([M, C], f32, tag="nat", name="k_nat")
            v_nat = natpool.tile([M, C], f32, tag="nat", name="v_nat")
            k_bf = big.tile([M, C], bf16, tag="k_bf", name="k_bf")
            kT = big.tile([P, CH, M], fp8, tag="kT", name="kT")

            def kv_proj(half):
                nat = k_nat if half == 0 else v_nat
                for j in range(2):
                    ps = pp.tile([M, 512], f32, tag=f"o{2 + j}",
                                 name=f"kv{half}{j}")
                    for ci in range(CH):
                        nc.tensor.matmul(
                            ps[:], xTb_sb[:, ci, :],
                            wkv_sb[:, half, ci, j * 512:(j + 1) * 512],
                            start=(ci == 0), stop=(ci == CH - 1),
                        )
                    sl = slice(j * 512, (j + 1) * 512)
                    if half == 0:
                        nc.scalar.copy(nat[:, sl], ps[:])
                        nc.vector.tensor_copy(out=k_bf[:, sl], in_=ps[:])
                    else:
                        nc.vector.tensor_add(out=vpad[0:M, sl], in0=ps[:],
                                             in1=bvb_sb[:, sl])
                        nc.vector.tensor_add(out=nat[:, sl], in0=ps[:],
                                             in1=bvb_sb[:, sl])
                if half == 0:
                    nc.scalar.dma_start(key_d[:], k_nat[:])
                else:
                    nc.scalar.dma_start(val_d[:], v_nat[:])

            def kT_piece():
                for ci in range(CH):
                    tp = pp.tile([P, P], f32, tag=f"t{ci % 2}", name="tpk")
                    nc.tensor.matmul(tp[:, 0:M], k_bf[:, ci * P:(ci + 1) * P],
                                     ident_b[0:M, 0:M], start=True, stop=True)
                    nc.vector.tensor_copy(out=kT[:, ci, :], in_=tp[:, 0:M])

            # ============ main schedule (consumption-order aligned) ========
            for w in range(NW):                      # scores(0) + T(0)
                scores_window(0, w)
                if w < NW - 1:
                    warm_fill(3)                     # keep PE busy: KT0
                    # arrives slower than warm-PE consumption
            scores_tail(0)
            sv_chunks(0, 0, 4)                       # <- V0s0
            scores_window(1, 0)                      # <- K1w01
            scores_window(1, 1)
            kv_proj(0)                               # <- Wk
            kT_piece()
            scores_newkey_a(0)
            sv_chunks(0, 4, 8)                       # <- V0s1
            scores_newkey_b(0)
            scores_window(1, 2)                      # <- K1w23
            scores_window(1, 3)
            sv_chunks(0, 8, 12)                      # <- V0s2
            scores_window(1, 4)                      # <- K1w45
            scores_window(1, 5)
            sv_chunks(0, 12, 16)                     # <- V0s3
            scores_window(1, 6)                      # <- K1w67
            scores_window(1, 7)
            scores_tail(1)
            scores_newkey_a(1)
            kv_proj(1)                               # <- Wv
            scores_newkey_b(1)
            sv_final(0)
            for ci in range(CH):
                gather_ci(0, ci)
            sv_chunks(1, 0, 4)                       # <- V1s0
            sv_chunks(1, 4, 8)
            sv_chunks(1, 8, 12)
            sv_chunks(1, 12, 16)
            sv_final(1)

            # gather(1) + out projection (DR), per-half so the first output
            # DMA overlaps the second half's matmuls            <- Wo
            ps_fin = [pp.tile([M, 512], f32, tag=f"s{j}", name=f"fin{j}")
                      for j in range(2)]
            fin = natpool.tile([M, C], f32, tag="nat", name="fin")
            for ci in range(CH):
                gather_ci(1, ci)
            for j in range(2):
                for pair in range(CH // 2):
                    ci = 2 * pair
                    nc.tensor.matmul(
                        ps_fin[j][:], wvT[:, ci:ci + 2, :],
                        wo_sb[:, ci:ci + 2, j * 512:(j + 1) * 512],
                        start=(pair == 0), stop=(pair == CH // 2 - 1),
                        perf_mode=DR,
                    )
                sl = slice(j * 512, (j + 1) * 512)
                nc.vector.tensor_add(out=fin[:, sl], in0=ps_fin[j][:],
                                     in1=bob_sb[:, sl])
                nc.scalar.dma_start(out_d[:, sl], fin[:, sl])

    nc.compile()
    return nc


def _prep_host(x, kv_cache, Wq, bq, Wk, Wv, bv, Wo, bo):
    fp8 = ml_dtypes.float8_e4m3
    bf16 = ml_dtypes.bfloat16
    f32 = np.float32
    x = np.asarray(x, f32)
    kv = np.asarray(kv_cache)
    Wq = np.asarray(Wq, f32); bq = np.asarray(bq, f32)
    Wk = np.asarray(Wk, f32); Wv = np.asarray(Wv, f32); bv = np.asarray(bv, f32)
    Wo = np.asarray(Wo, f32); bo = np.asarray(bo, f32)

    # K-cache / V-cache repacked so every device DMA is a fully contiguous
    # [128 x >=4KB] transfer:
    #   KT[b, w, p, ci*512 + j] = K[b, w*512 + j, ci*128 + p]
    #   Vd[b, s, p, tt*C + c]   = V[b, (s*NV + tt)*128 + p, c]
    KT_all = np.asarray(kv[:, 1, 0], f32).transpose(0, 2, 1).reshape(
        B, CH, P, NW, 512).transpose(0, 3, 2, 1, 4)
    KT_all = np.ascontiguousarray(KT_all).astype(fp8)
    V_all = np.asarray(kv[:, 1, 1], f32).reshape(
        B, L // (P * NV), NV, P, C).transpose(0, 1, 3, 2, 4)
    V_all = np.ascontiguousarray(V_all).astype(fp8)

    # weights: [P, CH, C(out)] with c_in = ci*128 + p
    Wq8 = np.ascontiguousarray(
        (Wq.T * SCALE).reshape(CH, P, C).transpose(1, 0, 2)).astype(fp8)
    Wo8 = np.ascontiguousarray(
        Wo.T.reshape(CH, P, C).transpose(1, 0, 2)).astype(fp8)
    Wkv8 = np.ascontiguousarray(np.stack([
        Wk.T.reshape(CH, P, C).transpose(1, 0, 2),
        Wv.T.reshape(CH, P, C).transpose(1, 0, 2)])).astype(bf16)
    bqs = np.ascontiguousarray((bq * SCALE).reshape(CH, P).T)  # [P, CH]
    bvb = np.ascontiguousarray(np.tile(bv, (M, 1)))
    bob = np.ascontiguousarray(np.tile(bo, (M, 1)))

    in_maps = []
    for c in range(NCORES):
        xc = x[c * BPC:(c + 1) * BPC].reshape(M, C)
        xT = np.ascontiguousarray(xc.reshape(M, CH, P).transpose(2, 1, 0))
        in_maps.append({
            "xT8": xT.astype(fp8),
            "xTb": xT.astype(bf16),
            "Wq8": Wq8.reshape(P, CH * C),
            "Wo8": Wo8.reshape(P, CH * C),
            "Wkvd": Wkv8.reshape(2, P, CH * C),
            "KT": np.ascontiguousarray(KT_all[c * BPC:(c + 1) * BPC]).reshape(
                BPC, NW, P, CH * 512),
            "Vd": np.ascontiguousarray(V_all[c * BPC:(c + 1) * BPC]).reshape(
                BPC, L // (P * NV), P, NV * C),
            "bqs": bqs, "bvb": bvb, "bob": bob,
        })
    return in_maps


def kernel(x, kv_cache, Wq, bq, Wk, Wv, bv, Wo, bo, _trace=False, _tmpdir=None):
    from concourse.bass_utils import run_bass_kernel_spmd

    _ensure_ntff_hook()
    if "nc" not in _CACHE:
        _CACHE["nc"] = _build()
    nc = _CACHE["nc"]

    in_maps = _prep_host(x, kv_cache, Wq, bq, Wk, Wv, bv, Wo, bo)
    res = run_bass_kernel_spmd(
        nc, in_maps, core_ids=list(range(NCORES)),
        trace=_trace, tmpdir=_tmpdir,
    )
    out = np.empty((B, T, C), np.float32)
    key_o = np.empty((B, T, C), np.float32)
    val_o = np.empty((B, T, C), np.float32)
    for c in range(NCORES):
        r = res.results[c]
        sl = slice(c * BPC, (c + 1) * BPC)
        out[sl] = r["out"].reshape(BPC, T, C)
        key_o[sl] = r["key"].reshape(BPC, T, C)
        val_o[sl] = r["value"].reshape(BPC, T, C)
    kernel._last_exec_time_ns = res.exec_time_ns
    kernel._last_results = res
    return (out, key_o, val_o)


# revision 26
# speedup vs baseline: 1.0681x; 1.0681x over previous
"""Trainium2 Bass kernel for cached multi-head self-attention decode step.

Problem (hardcoded):
  B=16, T=8, C=1024, n_head=16, head_dim=64, Lcache=4096, layer index 1.
  reference:
    q = x@Wq.T + bq ; key = x@Wk.T ; value = x@Wv.T + bv
    K = concat(kv_cache[:,1,0], key) ; V = concat(kv_cache[:,1,1], value)
    out = softmax((q*s)(K*s)^T) @ V @ Wo.T + bo      (s = hd**-0.25)
    returns (out, key, value)

Sharding: data-parallel over batch. 8 cores x 2 batches each. No collectives.

v4 design:
  - fp8 DoubleRow matmuls (contract 256/instr, measured ~225ns warm at
    N=512 - 2x bf16 per contract) for scores, S@V, q-proj, out-proj.
  - all transposes via matmul with identity rhs.
  - ONE ordered sync-queue DMA stream; PE program order aligned with DMA
    completion order so the PE never head-of-line blocks:
      x,Wq | KT0 | V0s0 | K1w01 | Wk | V0s1 | K1w23 | V0s2 | K1w45 |
      V0s3 | K1w67 | Wv | V1 | Wo
    PE: warmup, q, scores0+T0, sv0(c0-3), s1w01+T1, kproj+kT+newkey0,
      sv0(c4-7), s1w23, sv0(c8-11), s1w45, sv0(c12-15), s1w67, vproj,
      svfinal0, On0, newkey1, gather0, sv1, svfinal1, On1, gather1+outproj.
  - PE kept dense so the HAM clock gate stays at 8/8 (cold MMs are 2x).
"""

import sys
import types

import numpy as np
import ml_dtypes

# ---- hardcoded problem geometry ----
B, T, C = 16, 8, 1024
H, HD = 16, 64
L = 4096            # cached length
LT = L + T          # total keys
NCORES = 8
BPC = B // NCORES   # batches per core = 2
M = BPC * T         # queries per core = 16
P = 128
CH = C // P         # 8 c-chunks
NW = L // 512       # 8 score windows of 512
NV = 8              # V l-chunks (128 rows) per DMA tile (1MB transfers)
NDR = L // 256      # 16 DoubleRow l-pair chunks per batch
NWARM = 14          # PE warmup matmuls of N=512 (HAM un-throttle + stay busy
#                     until the first weights land ~13us in)
SCALE = float(HD) ** -0.5  # folded into Wq/bq on host

# softmax logit shift: exp(s + ESHIFT); cancels in normalization, keeps the
# fp8 S@V weights well inside e4m3 range.
ESHIFT = -2.0

_CACHE = {}


def _ensure_ntff_hook():
    """run_bass_kernel_spmd(trace=True) under axon needs antenv.axon_hooks;
    shim it from the boot module if the image's antenv lacks it."""
    try:
        import antenv.axon_hooks  # noqa: F401
        return
    except ImportError:
        pass
    try:
        import trn_agent_boot.trn_boot as tb
        hook = tb._ntff_profile_via_ctypes("/opt/axon/libaxon_pjrt.so")
    except Exception:
        hook = None
    mod = types.ModuleType("antenv.axon_hooks")
    mod.get_axon_ntff_profile_hook = lambda: hook
    mod.set_axon_ntff_profile_hook = lambda h: None
    sys.modules["antenv.axon_hooks"] = mod


def _build():
    import concourse.bacc as bacc
    import concourse.mybir as mybir
    import concourse.tile as tile
    from concourse.masks import make_identity

    f32 = mybir.dt.float32
    bf16 = mybir.dt.bfloat16
    fp8 = mybir.dt.float8e4
    DR = mybir.MatmulPerfMode.DoubleRow

    nc = bacc.Bacc(None, target_bir_lowering=False)

    # ---- dram I/O (all host-repacked for contiguous loads) ----
    xT8 = nc.dram_tensor("xT8", [P, CH, M], fp8, kind="ExternalInput")
    xTb = nc.dram_tensor("xTb", [P, CH, M], bf16, kind="ExternalInput")
    Wq8 = nc.dram_tensor("Wq8", [P, CH * C], fp8, kind="ExternalInput")
    Wo8 = nc.dram_tensor("Wo8", [P, CH * C], fp8, kind="ExternalInput")
    Wkvd = nc.dram_tensor("Wkvd", [2, P, CH * C], bf16, kind="ExternalInput")
    KT = nc.dram_tensor("KT", [BPC, NW, P, CH * 512], fp8, kind="ExternalInput")
    Vd = nc.dram_tensor("Vd", [BPC, L // (P * NV), P, NV * C], fp8,
                        kind="ExternalInput")
    bqs = nc.dram_tensor("bqs", [P, CH], f32, kind="ExternalInput")
    bvb = nc.dram_tensor("bvb", [M, C], f32, kind="ExternalInput")
    bob = nc.dram_tensor("bob", [M, C], f32, kind="ExternalInput")
    out_d = nc.dram_tensor("out", [M, C], f32, kind="ExternalOutput")
    key_d = nc.dram_tensor("key", [M, C], f32, kind="ExternalOutput")
    val_d = nc.dram_tensor("value", [M, C], f32, kind="ExternalOutput")

    AF = mybir.ActivationFunctionType
    AX = mybir.AxisListType
    OP = mybir.AluOpType

    with tile.TileContext(nc) as tc:
        with (
            tc.tile_pool(name="const", bufs=1) as cpool,
            tc.tile_pool(name="kt", bufs=15) as ktpool,
            tc.tile_pool(name="v", bufs=6) as vpool,
            tc.tile_pool(name="nat", bufs=2) as natpool,
            tc.tile_pool(name="wchunk", bufs=2 * NW + 2) as wtpool,
            tc.tile_pool(name="big", bufs=1) as big,
            tc.tile_pool(name="ps", bufs=1, space="PSUM") as pp,
        ):
            # ---------------- DMA issue order == consumption order --------
            xT8_sb = cpool.tile([P, CH, M], fp8, tag="xT8", name="xT8")
            nc.sync.dma_start(xT8_sb[:], xT8[:])
            xTb_sb = cpool.tile([P, CH, M], bf16, tag="xTb", name="xTb")
            nc.sync.dma_start(xTb_sb[:], xTb[:])
            wq_sb = cpool.tile([P, CH, C], fp8, tag="wq", name="wq")
            nc.sync.dma_start(wq_sb[:], Wq8[:])
            kts = {}

            def kt_dma(b, w):
                kts[(b, w)] = ktpool.tile([P, CH, 512], fp8, tag="kt",
                                          name="kt")
                nc.sync.dma_start(kts[(b, w)][:], KT[b, w])

            vts = {}

            def v_dma(b, s):
                vts[(b, s)] = vpool.tile([P, NV, C], fp8, tag="v", name="v")
                nc.sync.dma_start(vts[(b, s)][:], Vd[b, s])

            wkv_sb = cpool.tile([P, 2, CH, C], bf16, tag="wkv", name="wkv")

            for w in range(NW):
                kt_dma(0, w)
            v_dma(0, 0)
            kt_dma(1, 0); kt_dma(1, 1)
            nc.sync.dma_start(wkv_sb[:, 0], Wkvd[0])   # Wk
            v_dma(0, 1)
            kt_dma(1, 2); kt_dma(1, 3)
            v_dma(0, 2)
            kt_dma(1, 4); kt_dma(1, 5)
            v_dma(0, 3)
            kt_dma(1, 6); kt_dma(1, 7)
            nc.sync.dma_start(wkv_sb[:, 1], Wkvd[1])   # Wv
            for s in range(4):
                v_dma(1, s)
            wo_sb = cpool.tile([P, CH, C], fp8, tag="wo", name="wo")
            nc.sync.dma_start(wo_sb[:], Wo8[:])

            # ---- constants / memsets (gpsimd memsets BEFORE its slow
            # SWDGE dma emissions so the PE warmup input is ready early) ----
            warm_in = cpool.tile([P, 512], bf16, tag="warm_in", name="warm_in")
            nc.gpsimd.memset(warm_in[:], 0.25)
            eshift = cpool.tile([P, 1], f32, tag="eshift", name="eshift")
            nc.gpsimd.memset(eshift[:], ESHIFT)
            Qb = {}
            for b in range(BPC):
                Qb[b] = big.tile([P, CH, P], fp8, tag=f"Qbig{b}",
                                 name=f"Qbig{b}")
                nc.gpsimd.memset(Qb[b][:], 0.0)
            vpad = big.tile([P, C], fp8, tag="vpad", name="vpad")
            nc.gpsimd.memset(vpad[:], 0.0)
            wn_pad, wt32 = {}, {}
            for b in range(BPC):
                wn_pad[b] = big.tile([P, M], fp8, tag=f"wn_pad{b}",
                                     name=f"wn_pad{b}")
                nc.gpsimd.memset(wn_pad[b][:], 0.0)
                wt32[b] = big.tile([P, P], fp8, tag=f"wt32_{b}",
                                   name=f"wt32_{b}")
                nc.gpsimd.memset(wt32[b][:], 0.0)
            bqs_sb = cpool.tile([P, CH], f32, tag="bqs", name="bqs")
            nc.gpsimd.dma_start(bqs_sb[:], bqs[:])
            bvb_sb = cpool.tile([M, C], bf16, tag="bvb", name="bvb")
            nc.gpsimd.dma_start(bvb_sb[:], bvb[:])
            bob_sb = cpool.tile([M, C], bf16, tag="bob", name="bob")
            nc.gpsimd.dma_start(bob_sb[:], bob[:])

            ident = cpool.tile([P, P], f32, tag="ident", name="ident")
            make_identity(nc, ident)
            ident_b = cpool.tile([P, P], bf16, tag="ident_b", name="ident_b")
            nc.vector.tensor_copy(out=ident_b[:], in_=ident[:])
            ident_8 = cpool.tile([P, P], fp8, tag="ident_8", name="ident_8")
            nc.vector.tensor_copy(out=ident_8[:], in_=ident[:])

            # ---- PE warmup / filler: keep the HAM clock gate at 8/8 (cold
            # matmuls run at 1.2 vs 2.4 GHz). Each call allocates from the
            # t-tag ring so WAR deps are tracked.
            _wk = [0]

            def warm_fill(n, nfree=64):
                wm = pp.tile([P, 512], f32, tag=f"t{_wk[0] % 2}", name="warm")
                _wk[0] += 1
                for _ in range(n):
                    nc.tensor.matmul(wm[:, 0:nfree], warm_in[:, 0:P],
                                     warm_in[:, 0:nfree],
                                     start=True, stop=True)

            warm_fill(NWARM, nfree=512)

            # ---------------- per-batch state ------------------------------
            W_s, sums, rsum, ops_b, On = {}, {}, {}, {}, {}
            for b in range(BPC):
                W_s[b] = big.tile([P, LT], fp8, tag=f"W{b}", name=f"W{b}")
                sums[b] = big.tile([P, NW + 1], f32, tag=f"sums{b}",
                                   name=f"sums{b}")
                On[b] = big.tile([P, C], bf16, tag=f"On{b}", name=f"On{b}")
            wts = {0: {}, 1: {}}  # (b -> chunk -> wt tile)

            # ---------------- Phase A: q projection (fp8 DR) --------------
            q_bf = big.tile([M, C], bf16, tag="q_bf", name="q_bf")
            for j in range(2):
                qps = pp.tile([M, 512], f32, tag=f"s{j}", name=f"qps{j}")
                for ci in range(0, CH, 2):
                    nc.tensor.matmul(
                        qps[:], xT8_sb[:, ci:ci + 2, :],
                        wq_sb[:, ci:ci + 2, j * 512:(j + 1) * 512],
                        start=(ci == 0), stop=(ci == CH - 2), perf_mode=DR,
                    )
                nc.scalar.copy(q_bf[:, j * 512:(j + 1) * 512], qps[:])
            for co in range(CH):
                tpq = pp.tile([P, P], f32, tag=f"t{co % 2}", name="tpq")
                nc.tensor.matmul(
                    tpq[:, 0:M], q_bf[:, co * P:(co + 1) * P],
                    ident_b[0:M, 0:M], start=True, stop=True,
                )
                for b in range(BPC):
                    for j in range(2):
                        rows = slice(64 * j, 64 * (j + 1))
                        nc.scalar.activation(
                            Qb[b][rows, co, 16 * co + 8 * j:16 * co + 8 * j + 8],
                            tpq[rows, b * T:b * T + T],
                            AF.Identity, bias=bqs_sb[rows, co:co + 1],
                        )

            def w_transpose(b, t):
                """wt pair tile for DR l-chunk t of batch b (2 matmul-T)."""
                wt = wtpool.tile([P, 2, P], fp8, tag="wt", name="wt")
                for i in range(2):
                    tpw = pp.tile([P, P], f32, tag=f"t{(2 * t + i) % 2}",
                                  name="tpw")
                    nc.tensor.matmul(
                        tpw[:], W_s[b][:, (2 * t + i) * P:(2 * t + i + 1) * P],
                        ident_8[:], start=True, stop=True,
                    )
                    nc.vector.tensor_copy(out=wt[:, i, :], in_=tpw[:])
                wts[b][t] = wt

            def scores_window(b, lw, transpose=True):
                kt = kts[(b, lw)]
                sp = pp.tile([P, 512], f32, tag=f"s{lw % 2}", name="sp")
                for ci in range(0, CH, 2):
                    nc.tensor.matmul(
                        sp[:], Qb[b][:, ci:ci + 2, :], kt[:, ci:ci + 2, :],
                        start=(ci == 0), stop=(ci == CH - 2), perf_mode=DR,
                    )
                nc.scalar.activation(
                    W_s[b][:, lw * 512:(lw + 1) * 512], sp[:], AF.Exp,
                    bias=eshift[:, 0:1],
                    accum_out=sums[b][:, lw:lw + 1],
                )
                if transpose and lw > 0:
                    w_transpose(b, 2 * (lw - 1))
                    w_transpose(b, 2 * (lw - 1) + 1)

            def scores_tail(b):
                w_transpose(b, 2 * (NW - 1))
                w_transpose(b, 2 * (NW - 1) + 1)

            def sv_chunks(b, t0, t1):
                """pure DR S@V over l-pair chunks [t0, t1)."""
                if t0 == 0:
                    ops_b[b] = [pp.tile([P, 512], f32, tag=f"o{2 * b + j}",
                                        name=f"sv{b}{j}") for j in range(2)]
                ops = ops_b[b]
                for t_ in range(t0, t1):
                    vt = vts[(b, t_ // 4)]
                    tt = t_ % 4
                    for j in range(2):
                        nc.tensor.matmul(
                            ops[j][:], wts[b][t_][:],
                            vt[:, 2 * tt:2 * tt + 2, j * 512:(j + 1) * 512],
                            start=(t_ == 0), stop=False, perf_mode=DR,
                        )
                    del wts[b][t_]

            def scores_newkey_a(b):
                # new-key scores + normalization sums + wn_pad staging; the
                # DVE tail here runs under whatever PE work follows.
                spn = pp.tile([P, 512], f32, tag=f"s{b % 2}", name="spn")
                for ci in range(0, CH, 2):
                    nc.tensor.matmul(
                        spn[:, 0:T], Qb[b][:, ci:ci + 2, :],
                        kT[:, ci:ci + 2, b * T:(b + 1) * T],
                        start=(ci == 0), stop=(ci == CH - 2), perf_mode=DR,
                    )
                nc.scalar.activation(
                    W_s[b][:, L:LT], spn[:, 0:T], AF.Exp,
                    bias=eshift[:, 0:1], accum_out=sums[b][:, NW:NW + 1],
                )
                rs = big.tile([P, 1], f32, tag=f"rs{b}", name=f"rs{b}")
                nc.vector.tensor_reduce(out=rs[:], in_=sums[b][:],
                                        axis=AX.X, op=OP.add)
                rsum[b] = big.tile([P, 1], f32, tag=f"rsum{b}", name=f"rsum{b}")
                nc.vector.reciprocal(rsum[b][:], rs[:])
                nc.vector.tensor_copy(out=wn_pad[b][:, b * T:(b + 1) * T],
                                      in_=W_s[b][:, L:LT])

            def scores_newkey_b(b):
                tpn = pp.tile([P, P], f32, tag=f"t{b % 2}", name="tpn")
                nc.tensor.matmul(tpn[0:M, :], wn_pad[b][:], ident_8[:],
                                 start=True, stop=True)
                nc.vector.tensor_copy(out=wt32[b][0:M, :], in_=tpn[0:M, :])

            def sv_final(b):
                ops = ops_b[b]
                for j in range(2):
                    nc.tensor.matmul(
                        ops[j][:], wt32[b][:], vpad[:, j * 512:(j + 1) * 512],
                        start=False, stop=True,
                    )
                    nc.scalar.activation(
                        On[b][:, j * 512:(j + 1) * 512], ops[j][:], AF.Copy,
                        scale=rsum[b][:],
                    )

            wvT = big.tile([P, CH, M], fp8, tag="wvT", name="wvT")

            def gather_ci(b, ci):
                tp = pp.tile([P, P], f32, tag=f"t{ci % 2}", name="tpg")
                nc.tensor.matmul(tp[:], On[b][:, ci * P:(ci + 1) * P],
                                 ident_b[:], start=True, stop=True)
                nc.vector.tensor_copy(
                    out=wvT[0:64, ci, b * T:(b + 1) * T],
                    in_=tp[0:64, 16 * ci:16 * ci + 8])
                nc.vector.tensor_copy(
                    out=wvT[64:P, ci, b * T:(b + 1) * T],
                    in_=tp[64:P, 16 * ci + 8:16 * ci + 16])

            # ---- k/v natural projection pieces (bf16) ----
            k_nat = natpool.tile([M, C], f32, tag="nat", name="k_nat")
            v_nat = natpool.tile([M, C], f32, tag="nat", name="v_nat")
            k_bf = big.tile([M, C], bf16, tag="k_bf", name="k_bf")
            kT = big.tile([P, CH, M], fp8, tag="kT", name="kT")

            def kv_proj(half):
                nat = k_nat if half == 0 else v_nat
                for j in range(2):
                    ps = pp.tile([M, 512], f32, tag=f"o{2 + j}",
                                 name=f"kv{half}{j}")
                    for ci in range(CH):
                        nc.tensor.matmul(
                            ps[:], xTb_sb[:, ci, :],
                            wkv_sb[:, half, ci, j * 512:(j + 1) * 512],
                            start=(ci == 0), stop=(ci == CH - 1),
                        )
                    sl = slice(j * 512, (j + 1) * 512)
                    if half == 0:
                        nc.scalar.copy(nat[:, sl], ps[:])
                        nc.vector.tensor_copy(out=k_bf[:, sl], in_=ps[:])
                    else:
                        nc.vector.tensor_add(out=vpad[0:M, sl], in0=ps[:],
                                             in1=bvb_sb[:, sl])
                        nc.vector.tensor_add(out=nat[:, sl], in0=ps[:],
                                             in1=bvb_sb[:, sl])
                if half == 0:
                    nc.scalar.dma_start(key_d[:], k_nat[:])
                else:
                    nc.scalar.dma_start(val_d[:], v_nat[:])

            def kT_piece():
                for ci in range(CH):
                    tp = pp.tile([P, P], f32, tag=f"t{ci % 2}", name="tpk")
                    nc.tensor.matmul(tp[:, 0:M], k_bf[:, ci * P:(ci + 1) * P],
                                     ident_b[0:M, 0:M], start=True, stop=True)
                    nc.vector.tensor_copy(out=kT[:, ci, :], in_=tp[:, 0:M])

            # ============ main schedule (consumption-order aligned) ========
            for w in range(NW):                      # scores(0) + T(0)
                scores_window(0, w)
            scores_tail(0)
            sv_chunks(0, 0, 4)                       # <- V0s0
            scores_window(1, 0)                      # <- K1w01
            scores_window(1, 1)
            kv_proj(0)                               # <- Wk
            kT_piece()
            scores_newkey_a(0)
            sv_chunks(0, 4, 8)                       # <- V0s1
            scores_newkey_b(0)
            scores_window(1, 2)                      # <- K1w23
            scores_window(1, 3)
            sv_chunks(0, 8, 12)                      # <- V0s2
            scores_window(1, 4)                      # <- K1w45
            scores_window(1, 5)
            sv_chunks(0, 12, 16)                     # <- V0s3
            scores_window(1, 6)                      # <- K1w67
            scores_window(1, 7)
            scores_tail(1)
            scores_newkey_a(1)
            kv_proj(1)                               # <- Wv
            scores_newkey_b(1)
            sv_final(0)
            for ci in range(CH):
                gather_ci(0, ci)
            sv_chunks(1, 0, 4)                       # <- V1s0
            sv_chunks(1, 4, 8)
            sv_chunks(1, 8, 12)
            sv_chunks(1, 12, 16)
            sv_final(1)

            # gather(1) + out projection (DR), per-half so the first output
            # DMA overlaps the second half's matmuls            <- Wo
            ps_fin = [pp.tile([M, 512], f32, tag=f"s{j}", name=f"fin{j}")
                      for j in range(2)]
            fin = natpool.tile([M, C], f32, tag="nat", name="fin")
            for ci in range(CH):
                gather_ci(1, ci)
            for j in range(2):
                for pair in range(CH // 2):
                    ci = 2 * pair
                    nc.tensor.matmul(
                        ps_fin[j][:], wvT[:, ci:ci + 2, :],
                        wo_sb[:, ci:ci + 2, j * 512:(j + 1) * 512],
                        start=(pair == 0), stop=(pair == CH // 2 - 1),
                        perf_mode=DR,
                    )
                sl = slice(j * 512, (j + 1) * 512)
                nc.vector.tensor_add(out=fin[:, sl], in0=ps_fin[j][:],
                                     in1=bob_sb[:, sl])
                nc.scalar.dma_start(out_d[:, sl], fin[:, sl])

    nc.compile()
    return nc


def _prep_host(x, kv_cache, Wq, bq, Wk, Wv, bv, Wo, bo):
    fp8 = ml_dtypes.float8_e4m3
    bf16 = ml_dtypes.bfloat16
    f32 = np.float32
    x = np.asarray(x, f32)
    kv = np.asarray(kv_cache)
    Wq = np.asarray(Wq, f32); bq = np.asarray(bq, f32)
    Wk = np.asarray(Wk, f32); Wv = np.asarray(Wv, f32); bv = np.asarray(bv, f32)
    Wo = np.asarray(Wo, f32); bo = np.asarray(bo, f32)

    # K-cache / V-cache repacked so every device DMA is a fully contiguous
    # [128 x >=4KB] transfer:
    #   KT[b, w, p, ci*512 + j] = K[b, w*512 + j, ci*128 + p]
    #   Vd[b, s, p, tt*C + c]   = V[b, (s*NV + tt)*128 + p, c]
    KT_all = np.asarray(kv[:, 1, 0], f32).transpose(0, 2, 1).reshape(
        B, CH, P, NW, 512).transpose(0, 3, 2, 1, 4)
    KT_all = np.ascontiguousarray(KT_all).astype(fp8)
    V_all = np.asarray(kv[:, 1, 1], f32).reshape(
        B, L // (P * NV), NV, P, C).transpose(0, 1, 3, 2, 4)
    V_all = np.ascontiguousarray(V_all).astype(fp8)

    # weights: [P, CH, C(out)] with c_in = ci*128 + p
    Wq8 = np.ascontiguousarray(
        (Wq.T * SCALE).reshape(CH, P, C).transpose(1, 0, 2)).astype(fp8)
    Wo8 = np.ascontiguousarray(
        Wo.T.reshape(CH, P, C).transpose(1, 0, 2)).astype(fp8)
    Wkv8 = np.ascontiguousarray(np.stack([
        Wk.T.reshape(CH, P, C).transpose(1, 0, 2),
        Wv.T.reshape(CH, P, C).transpose(1, 0, 2)])).astype(bf16)
    bqs = np.ascontiguousarray((bq * SCALE).reshape(CH, P).T)  # [P, CH]
    bvb = np.ascontiguousarray(np.tile(bv, (M, 1)))
    bob = np.ascontiguousarray(np.tile(bo, (M, 1)))

    in_maps = []
    for c in range(NCORES):
        xc = x[c * BPC:(c + 1) * BPC].reshape(M, C)
        xT = np.ascontiguousarray(xc.reshape(M, CH, P).transpose(2, 1, 0))
        in_maps.append({
            "xT8": xT.astype(fp8),
            "xTb": xT.astype(bf16),
            "Wq8": Wq8.reshape(P, CH * C),
            "Wo8": Wo8.reshape(P, CH * C),
            "Wkvd": Wkv8.reshape(2, P, CH * C),
            "KT": np.ascontiguousarray(KT_all[c * BPC:(c + 1) * BPC]).reshape(
                BPC, NW, P, CH * 512),
            "Vd": np.ascontiguousarray(V_all[c * BPC:(c + 1) * BPC]).reshape(
                BPC, L // (P * NV), P, NV * C),
            "bqs": bqs, "bvb": bvb, "bob": bob,
        })
    return in_maps


def kernel(x, kv_cache, Wq, bq, Wk, Wv, bv, Wo, bo, _trace=False, _tmpdir=None):
    from concourse.bass_utils import run_bass_kernel_spmd

    _ensure_ntff_hook()
    if "nc" not in _CACHE:
        _CACHE["nc"] = _build()
    nc = _CACHE["nc"]

    in_maps = _prep_host(x, kv_cache, Wq, bq, Wk, Wv, bv, Wo, bo)
    res = run_bass_kernel_spmd(
        nc, in_maps, core_ids=list(range(NCORES)),
        trace=_trace, tmpdir=_tmpdir,
    )
    out = np.empty((B, T, C), np.float32)
    key_o = np.empty((B, T, C), np.float32)
    val_o = np.empty((B, T, C), np.float32)
    for c in range(NCORES):
        r = res.results[c]
        sl = slice(c * BPC, (c + 1) * BPC)
        out[sl] = r["out"].reshape(BPC, T, C)
        key_o[sl] = r["key"].reshape(BPC, T, C)
        val_o[sl] = r["value"].reshape(BPC, T, C)
    kernel._last_exec_time_ns = res.exec_time_ns
    kernel._last_results = res
    return (out, key_o, val_o)


# revision 34
# speedup vs baseline: 1.0755x; 1.0070x over previous
"""Trainium2 Bass kernel for cached multi-head self-attention decode step.

Problem (hardcoded):
  B=16, T=8, C=1024, n_head=16, head_dim=64, Lcache=4096, layer index 1.
  reference:
    q = x@Wq.T + bq ; key = x@Wk.T ; value = x@Wv.T + bv
    K = concat(kv_cache[:,1,0], key) ; V = concat(kv_cache[:,1,1], value)
    out = softmax((q*s)(K*s)^T) @ V @ Wo.T + bo      (s = hd**-0.25)
    returns (out, key, value)

Sharding: data-parallel over batch. 8 cores x 2 batches each. No collectives.

v4 design:
  - fp8 DoubleRow matmuls (contract 256/instr, measured ~225ns warm at
    N=512 - 2x bf16 per contract) for scores, S@V, q-proj, out-proj.
  - all transposes via matmul with identity rhs.
  - ONE ordered sync-queue DMA stream; PE program order aligned with DMA
    completion order so the PE never head-of-line blocks:
      x,Wq | KT0 | V0s0 | K1w01 | Wk | V0s1 | K1w23 | V0s2 | K1w45 |
      V0s3 | K1w67 | Wv | V1 | Wo
    PE: warmup, q, scores0+T0, sv0(c0-3), s1w01+T1, kproj+kT+newkey0,
      sv0(c4-7), s1w23, sv0(c8-11), s1w45, sv0(c12-15), s1w67, vproj,
      svfinal0, On0, newkey1, gather0, sv1, svfinal1, On1, gather1+outproj.
  - PE kept dense so the HAM clock gate stays at 8/8 (cold MMs are 2x).
"""

import sys
import types

import numpy as np
import ml_dtypes

# ---- hardcoded problem geometry ----
B, T, C = 16, 8, 1024
H, HD = 16, 64
L = 4096            # cached length
LT = L + T          # total keys
NCORES = 8
BPC = B // NCORES   # batches per core = 2
M = BPC * T         # queries per core = 16
P = 128
CH = C // P         # 8 c-chunks
NW = L // 512       # 8 score windows of 512
NV = 8              # V l-chunks (128 rows) per DMA tile (1MB transfers)
NDR = L // 256      # 16 DoubleRow l-pair chunks per batch
NWARM = 14          # PE warmup matmuls of N=512 (HAM un-throttle + stay busy
#                     until the first weights land ~13us in)
SCALE = float(HD) ** -0.5  # folded into Wq/bq on host

# softmax logit shift: exp(s + ESHIFT); cancels in normalization, keeps the
# fp8 S@V weights well inside e4m3 range.
ESHIFT = -2.0

_CACHE = {}


def _ensure_ntff_hook():
    """run_bass_kernel_spmd(trace=True) under axon needs antenv.axon_hooks;
    shim it from the boot module if the image's antenv lacks it."""
    try:
        import antenv.axon_hooks  # noqa: F401
        return
    except ImportError:
        pass
    try:
        import trn_agent_boot.trn_boot as tb
        hook = tb._ntff_profile_via_ctypes("/opt/axon/libaxon_pjrt.so")
    except Exception:
        hook = None
    mod = types.ModuleType("antenv.axon_hooks")
    mod.get_axon_ntff_profile_hook = lambda: hook
    mod.set_axon_ntff_profile_hook = lambda h: None
    sys.modules["antenv.axon_hooks"] = mod


def _build():
    import concourse.bacc as bacc
    import concourse.mybir as mybir
    import concourse.tile as tile
    from concourse.masks import make_identity

    f32 = mybir.dt.float32
    bf16 = mybir.dt.bfloat16
    fp8 = mybir.dt.float8e4
    DR = mybir.MatmulPerfMode.DoubleRow

    nc = bacc.Bacc(None, target_bir_lowering=False)

    # ---- dram I/O (all host-repacked for contiguous loads) ----
    xT8 = nc.dram_tensor("xT8", [P, CH, M], fp8, kind="ExternalInput")
    Wq8 = nc.dram_tensor("Wq8", [P, CH * C], fp8, kind="ExternalInput")
    Wo8 = nc.dram_tensor("Wo8", [P, CH * C], fp8, kind="ExternalInput")
    Wkv8 = nc.dram_tensor("Wkv8", [2, P, CH * C], fp8, kind="ExternalInput")
    xTall = nc.dram_tensor("xTall", [P, CH, B * T], bf16,
                           kind="ExternalInput")
    Wsl = nc.dram_tensor("Wsl", [P, CH * 2 * P], bf16, kind="ExternalInput")
    bvsl = nc.dram_tensor("bvsl", [B * T, P], f32, kind="ExternalInput")
    KT = nc.dram_tensor("KT", [BPC, NW, P, CH * 512], fp8, kind="ExternalInput")
    Vd = nc.dram_tensor("Vd", [BPC, L // (P * NV), P, NV * C], fp8,
                        kind="ExternalInput")
    bqs = nc.dram_tensor("bqs", [P, CH], f32, kind="ExternalInput")
    bvb = nc.dram_tensor("bvb", [M, C], f32, kind="ExternalInput")
    bob = nc.dram_tensor("bob", [M, C], f32, kind="ExternalInput")
    out_d = nc.dram_tensor("out", [M, C], f32, kind="ExternalOutput")
    ksl_d = nc.dram_tensor("key_sl", [B * T, P], f32, kind="ExternalOutput")
    vsl_d = nc.dram_tensor("val_sl", [B * T, P], f32, kind="ExternalOutput")

    AF = mybir.ActivationFunctionType
    AX = mybir.AxisListType
    OP = mybir.AluOpType

    with tile.TileContext(nc) as tc:
        with (
            tc.tile_pool(name="const", bufs=1) as cpool,
            tc.tile_pool(name="kt", bufs=15) as ktpool,
            tc.tile_pool(name="v", bufs=6) as vpool,
            tc.tile_pool(name="nat", bufs=2) as natpool,
            tc.tile_pool(name="wchunk", bufs=2 * NW + 2) as wtpool,
            tc.tile_pool(name="big", bufs=1) as big,
            tc.tile_pool(name="ps", bufs=1, space="PSUM") as pp,
        ):
            # ---------------- DMA issue order == consumption order --------
            xT8_sb = cpool.tile([P, CH, M], fp8, tag="xT8", name="xT8")
            nc.sync.dma_start(xT8_sb[:], xT8[:])
            wq_sb = cpool.tile([P, CH, C], fp8, tag="wq", name="wq")
            nc.sync.dma_start(wq_sb[:], Wq8[:])
            kts = {}

            def kt_dma(b, w):
                kts[(b, w)] = ktpool.tile([P, CH, 512], fp8, tag="kt",
                                          name="kt")
                nc.sync.dma_start(kts[(b, w)][:], KT[b, w])

            vts = {}

            def v_dma(b, s):
                vts[(b, s)] = vpool.tile([P, NV, C], fp8, tag="v", name="v")
                nc.sync.dma_start(vts[(b, s)][:], Vd[b, s])

            wkv_sb = cpool.tile([P, 2, CH, C], fp8, tag="wkv", name="wkv")
            xall_sb = cpool.tile([P, CH, B * T], bf16, tag="xall", name="xall")
            wsl_sb = cpool.tile([P, CH, 2 * P], bf16, tag="wsl", name="wsl")
            bvsl_sb = cpool.tile([B * T, P], f32, tag="bvsl", name="bvsl")

            for w in range(NW):
                kt_dma(0, w)
            v_dma(0, 0)
            kt_dma(1, 0); kt_dma(1, 1)
            nc.sync.dma_start(wkv_sb[:, 0], Wkv8[0])   # Wk fp8
            v_dma(0, 1)
            kt_dma(1, 2); kt_dma(1, 3)
            nc.sync.dma_start(xall_sb[:], xTall[:])
            nc.sync.dma_start(wsl_sb[:], Wsl[:])
            nc.sync.dma_start(bvsl_sb[:], bvsl[:])
            v_dma(0, 2)
            kt_dma(1, 4); kt_dma(1, 5)
            v_dma(0, 3)
            kt_dma(1, 6); kt_dma(1, 7)
            nc.sync.dma_start(wkv_sb[:, 1], Wkv8[1])   # Wv fp8
            for s in range(4):
                v_dma(1, s)
            wo_sb = cpool.tile([P, CH, C], fp8, tag="wo", name="wo")
            nc.sync.dma_start(wo_sb[:], Wo8[:])

            # ---- constants / memsets (gpsimd memsets BEFORE its slow
            # SWDGE dma emissions so the PE warmup input is ready early) ----
            warm_in = cpool.tile([P, 512], bf16, tag="warm_in", name="warm_in")
            nc.gpsimd.memset(warm_in[:], 0.25)
            eshift = cpool.tile([P, 1], f32, tag="eshift", name="eshift")
            nc.gpsimd.memset(eshift[:], ESHIFT)
            Qb = {}
            for b in range(BPC):
                Qb[b] = big.tile([P, CH, P], fp8, tag=f"Qbig{b}",
                                 name=f"Qbig{b}")
                nc.gpsimd.memset(Qb[b][:], 0.0)
            vpad = big.tile([P, C], fp8, tag="vpad", name="vpad")
            nc.gpsimd.memset(vpad[:], 0.0)
            wn_pad, wt32 = {}, {}
            for b in range(BPC):
                wn_pad[b] = big.tile([P, M], fp8, tag=f"wn_pad{b}",
                                     name=f"wn_pad{b}")
                nc.gpsimd.memset(wn_pad[b][:], 0.0)
                wt32[b] = big.tile([P, P], fp8, tag=f"wt32_{b}",
                                   name=f"wt32_{b}")
                nc.gpsimd.memset(wt32[b][:], 0.0)
            bqs_sb = cpool.tile([P, CH], f32, tag="bqs", name="bqs")
            nc.gpsimd.dma_start(bqs_sb[:], bqs[:])
            bvb_sb = cpool.tile([M, C], bf16, tag="bvb", name="bvb")
            nc.gpsimd.dma_start(bvb_sb[:], bvb[:])
            bob_sb = cpool.tile([M, C], bf16, tag="bob", name="bob")
            nc.gpsimd.dma_start(bob_sb[:], bob[:])

            ident = cpool.tile([P, P], f32, tag="ident", name="ident")
            make_identity(nc, ident)
            ident_b = cpool.tile([P, P], bf16, tag="ident_b", name="ident_b")
            nc.vector.tensor_copy(out=ident_b[:], in_=ident[:])
            ident_8 = cpool.tile([P, P], fp8, tag="ident_8", name="ident_8")
            nc.vector.tensor_copy(out=ident_8[:], in_=ident[:])

            # ---- PE warmup / filler: keep the HAM clock gate at 8/8 (cold
            # matmuls run at 1.2 vs 2.4 GHz). Each call allocates from the
            # t-tag ring so WAR deps are tracked.
            _wk = [0]

            def warm_fill(n, nfree=64):
                wm = pp.tile([P, 512], f32, tag=f"t{_wk[0] % 2}", name="warm")
                _wk[0] += 1
                for _ in range(n):
                    nc.tensor.matmul(wm[:, 0:nfree], warm_in[:, 0:P],
                                     warm_in[:, 0:nfree],
                                     start=True, stop=True)

            warm_fill(NWARM, nfree=512)

            # ---------------- per-batch state ------------------------------
            W_s, sums, rsum, ops_b, On = {}, {}, {}, {}, {}
            for b in range(BPC):
                W_s[b] = big.tile([P, LT], fp8, tag=f"W{b}", name=f"W{b}")
                sums[b] = big.tile([P, NW + 1], f32, tag=f"sums{b}",
                                   name=f"sums{b}")
                On[b] = big.tile([P, C], bf16, tag=f"On{b}", name=f"On{b}")
            wts = {0: {}, 1: {}}  # (b -> chunk -> wt tile)

            # ---------------- Phase A: q projection (fp8 DR) --------------
            q_bf = big.tile([M, C], bf16, tag="q_bf", name="q_bf")
            for j in range(2):
                qps = pp.tile([M, 512], f32, tag=f"s{j}", name=f"qps{j}")
                for ci in range(0, CH, 2):
                    nc.tensor.matmul(
                        qps[:], xT8_sb[:, ci:ci + 2, :],
                        wq_sb[:, ci:ci + 2, j * 512:(j + 1) * 512],
                        start=(ci == 0), stop=(ci == CH - 2), perf_mode=DR,
                    )
                nc.scalar.copy(q_bf[:, j * 512:(j + 1) * 512], qps[:])
            for co in range(CH):
                tpq = pp.tile([P, P], f32, tag=f"t{co % 2}", name="tpq")
                nc.tensor.matmul(
                    tpq[:, 0:M], q_bf[:, co * P:(co + 1) * P],
                    ident_b[0:M, 0:M], start=True, stop=True,
                )
                for b in range(BPC):
                    for j in range(2):
                        rows = slice(64 * j, 64 * (j + 1))
                        nc.scalar.activation(
                            Qb[b][rows, co, 16 * co + 8 * j:16 * co + 8 * j + 8],
                            tpq[rows, b * T:b * T + T],
                            AF.Identity, bias=bqs_sb[rows, co:co + 1],
                        )

            def w_transpose(b, t):
                """wt pair tile for DR l-chunk t of batch b (2 matmul-T)."""
                wt = wtpool.tile([P, 2, P], fp8, tag="wt", name="wt")
                for i in range(2):
                    tpw = pp.tile([P, P], f32, tag=f"t{(2 * t + i) % 2}",
                                  name="tpw")
                    nc.tensor.matmul(
                        tpw[:], W_s[b][:, (2 * t + i) * P:(2 * t + i + 1) * P],
                        ident_8[:], start=True, stop=True,
                    )
                    nc.vector.tensor_copy(out=wt[:, i, :], in_=tpw[:])
                wts[b][t] = wt

            def scores_window(b, lw, transpose=True):
                kt = kts[(b, lw)]
                sp = pp.tile([P, 512], f32, tag=f"s{lw % 2}", name="sp")
                for ci in range(0, CH, 2):
                    nc.tensor.matmul(
                        sp[:], Qb[b][:, ci:ci + 2, :], kt[:, ci:ci + 2, :],
                        start=(ci == 0), stop=(ci == CH - 2), perf_mode=DR,
                    )
                nc.scalar.activation(
                    W_s[b][:, lw * 512:(lw + 1) * 512], sp[:], AF.Exp,
                    bias=eshift[:, 0:1],
                    accum_out=sums[b][:, lw:lw + 1],
                )
                if transpose and lw > 0:
                    w_transpose(b, 2 * (lw - 1))
                    w_transpose(b, 2 * (lw - 1) + 1)

            def scores_tail(b):
                w_transpose(b, 2 * (NW - 1))
                w_transpose(b, 2 * (NW - 1) + 1)

            def sv_chunks(b, t0, t1):
                """pure DR S@V over l-pair chunks [t0, t1)."""
                if t0 == 0:
                    ops_b[b] = [pp.tile([P, 512], f32, tag=f"o{2 * b + j}",
                                        name=f"sv{b}{j}") for j in range(2)]
                ops = ops_b[b]
                for t_ in range(t0, t1):
                    vt = vts[(b, t_ // 4)]
                    tt = t_ % 4
                    for j in range(2):
                        nc.tensor.matmul(
                            ops[j][:], wts[b][t_][:],
                            vt[:, 2 * tt:2 * tt + 2, j * 512:(j + 1) * 512],
                            start=(t_ == 0), stop=False, perf_mode=DR,
                        )
                    del wts[b][t_]

            def scores_newkey_a(b):
                # new-key scores + normalization sums + wn_pad staging; the
                # DVE tail here runs under whatever PE work follows.
                spn = pp.tile([P, 512], f32, tag=f"s{b % 2}", name="spn")
                for ci in range(0, CH, 2):
                    nc.tensor.matmul(
                        spn[:, 0:T], Qb[b][:, ci:ci + 2, :],
                        kT[:, ci:ci + 2, b * T:(b + 1) * T],
                        start=(ci == 0), stop=(ci == CH - 2), perf_mode=DR,
                    )
                nc.scalar.activation(
                    W_s[b][:, L:LT], spn[:, 0:T], AF.Exp,
                    bias=eshift[:, 0:1], accum_out=sums[b][:, NW:NW + 1],
                )
                rs = big.tile([P, 1], f32, tag=f"rs{b}", name=f"rs{b}")
                nc.vector.tensor_reduce(out=rs[:], in_=sums[b][:],
                                        axis=AX.X, op=OP.add)
                rsum[b] = big.tile([P, 1], f32, tag=f"rsum{b}", name=f"rsum{b}")
                nc.vector.reciprocal(rsum[b][:], rs[:])
                nc.vector.tensor_copy(out=wn_pad[b][:, b * T:(b + 1) * T],
                                      in_=W_s[b][:, L:LT])

            def scores_newkey_b(b):
                tpn = pp.tile([P, P], f32, tag=f"t{b % 2}", name="tpn")
                nc.tensor.matmul(tpn[0:M, :], wn_pad[b][:], ident_8[:],
                                 start=True, stop=True)
                nc.vector.tensor_copy(out=wt32[b][0:M, :], in_=tpn[0:M, :])

            def sv_final(b):
                ops = ops_b[b]
                for j in range(2):
                    nc.tensor.matmul(
                        ops[j][:], wt32[b][:], vpad[:, j * 512:(j + 1) * 512],
                        start=False, stop=True,
                    )
                    nc.scalar.activation(
                        On[b][:, j * 512:(j + 1) * 512], ops[j][:], AF.Copy,
                        scale=rsum[b][:],
                    )

            wvT = big.tile([P, CH, M], fp8, tag="wvT", name="wvT")

            def gather_ci(b, ci):
                tp = pp.tile([P, P], f32, tag=f"t{ci % 2}", name="tpg")
                nc.tensor.matmul(tp[:], On[b][:, ci * P:(ci + 1) * P],
                                 ident_b[:], start=True, stop=True)
                nc.vector.tensor_copy(
                    out=wvT[0:64, ci, b * T:(b + 1) * T],
                    in_=tp[0:64, 16 * ci:16 * ci + 8])
                nc.vector.tensor_copy(
                    out=wvT[64:P, ci, b * T:(b + 1) * T],
                    in_=tp[64:P, 16 * ci + 8:16 * ci + 16])

            # ---- k/v projections ----
            # attention-grade (fp8 DR) k/v for the core's own 2 batches;
            # output-grade (bf16) key/value c-slices for ALL batches.
            k_bf = big.tile([M, C], bf16, tag="k_bf", name="k_bf")
            kT = big.tile([P, CH, M], fp8, tag="kT", name="kT")
            ksl = natpool.tile([B * T, P], f32, tag="nat", name="ksl")
            vsl = natpool.tile([B * T, P], f32, tag="nat", name="vsl")

            def kv_proj(half):
                for j in range(2):
                    ps = pp.tile([M, 512], f32, tag=f"o{2 + j}",
                                 name=f"kv{half}{j}")
                    for ci in range(0, CH, 2):
                        nc.tensor.matmul(
                            ps[:], xT8_sb[:, ci:ci + 2, :],
                            wkv_sb[:, half, ci:ci + 2, j * 512:(j + 1) * 512],
                            start=(ci == 0), stop=(ci == CH - 2), perf_mode=DR,
                        )
                    sl = slice(j * 512, (j + 1) * 512)
                    if half == 0:
                        nc.vector.tensor_copy(out=k_bf[:, sl], in_=ps[:])
                    else:
                        nc.vector.tensor_add(out=vpad[0:M, sl], in0=ps[:],
                                             in1=bvb_sb[:, sl])

            def slice_proj():
                # key/value output c-slice for ALL 16 batches (bf16 grade)
                ps = pp.tile([B * T, 2 * P], f32, tag="o2", name="psl")
                for ci in range(CH):
                    nc.tensor.matmul(
                        ps[:], xall_sb[:, ci, :], wsl_sb[:, ci, :],
                        start=(ci == 0), stop=(ci == CH - 1),
                    )
                nc.scalar.copy(ksl[:], ps[:, 0:P])
                nc.vector.tensor_add(out=vsl[:], in0=ps[:, P:2 * P],
                                     in1=bvsl_sb[:])
                nc.scalar.dma_start(ksl_d[:], ksl[:])
                nc.scalar.dma_start(vsl_d[:], vsl[:])

            def kT_piece():
                for ci in range(CH):
                    tp = pp.tile([P, P], f32, tag=f"t{ci % 2}", name="tpk")
                    nc.tensor.matmul(tp[:, 0:M], k_bf[:, ci * P:(ci + 1) * P],
                                     ident_b[0:M, 0:M], start=True, stop=True)
                    nc.vector.tensor_copy(out=kT[:, ci, :], in_=tp[:, 0:M])

            # ============ main schedule (consumption-order aligned) ========
            for w in range(NW):                      # scores(0) + T(0)
                scores_window(0, w)
            scores_tail(0)
            sv_chunks(0, 0, 4)                       # <- V0s0
            scores_window(1, 0)                      # <- K1w01
            scores_window(1, 1)
            kv_proj(0)                               # <- Wk
            kT_piece()
            scores_newkey_a(0)
            sv_chunks(0, 4, 8)                       # <- V0s1
            scores_newkey_b(0)
            scores_window(1, 2)                      # <- K1w23
            scores_window(1, 3)
            slice_proj()                             # <- Wsl/xTall
            sv_chunks(0, 8, 12)                      # <- V0s2
            scores_window(1, 4)                      # <- K1w45
            scores_window(1, 5)
            sv_chunks(0, 12, 16)                     # <- V0s3
            scores_window(1, 6)                      # <- K1w67
            scores_window(1, 7)
            scores_tail(1)
            scores_newkey_a(1)
            kv_proj(1)                               # <- Wv
            scores_newkey_b(1)
            sv_final(0)
            for ci in range(CH):
                gather_ci(0, ci)
            sv_chunks(1, 0, 4)                       # <- V1s0
            sv_chunks(1, 4, 8)
            sv_chunks(1, 8, 12)
            sv_chunks(1, 12, 16)
            sv_final(1)

            # gather(1) + out projection (DR), 1-pair chase     <- Wo
            ps_fin = [pp.tile([M, 512], f32, tag=f"s{j}", name=f"fin{j}")
                      for j in range(2)]
            fin = natpool.tile([M, C], f32, tag="nat", name="fin")
            gather_ci(1, 0)
            gather_ci(1, 1)
            for pair in range(CH // 2):
                if pair < 3:
                    gather_ci(1, 2 * pair + 2)
                    gather_ci(1, 2 * pair + 3)
                ci = 2 * pair
                for j in range(2):
                    nc.tensor.matmul(
                        ps_fin[j][:], wvT[:, ci:ci + 2, :],
                        wo_sb[:, ci:ci + 2, j * 512:(j + 1) * 512],
                        start=(pair == 0), stop=(pair == CH // 2 - 1),
                        perf_mode=DR,
                    )
            for j in range(2):
                sl = slice(j * 512, (j + 1) * 512)
                nc.vector.tensor_add(out=fin[:, sl], in0=ps_fin[j][:],
                                     in1=bob_sb[:, sl])
                nc.scalar.dma_start(out_d[:, sl], fin[:, sl])

    nc.compile()
    return nc


def _prep_host(x, kv_cache, Wq, bq, Wk, Wv, bv, Wo, bo):
    fp8 = ml_dtypes.float8_e4m3
    bf16 = ml_dtypes.bfloat16
    f32 = np.float32
    x = np.asarray(x, f32)
    kv = np.asarray(kv_cache)
    Wq = np.asarray(Wq, f32); bq = np.asarray(bq, f32)
    Wk = np.asarray(Wk, f32); Wv = np.asarray(Wv, f32); bv = np.asarray(bv, f32)
    Wo = np.asarray(Wo, f32); bo = np.asarray(bo, f32)

    # K-cache / V-cache repacked so every device DMA is a fully contiguous
    # [128 x >=4KB] transfer:
    #   KT[b, w, p, ci*512 + j] = K[b, w*512 + j, ci*128 + p]
    #   Vd[b, s, p, tt*C + c]   = V[b, (s*NV + tt)*128 + p, c]
    KT_all = np.asarray(kv[:, 1, 0], f32).transpose(0, 2, 1).reshape(
        B, CH, P, NW, 512).transpose(0, 3, 2, 1, 4)
    KT_all = np.ascontiguousarray(KT_all).astype(fp8)
    V_all = np.asarray(kv[:, 1, 1], f32).reshape(
        B, L // (P * NV), NV, P, C).transpose(0, 1, 3, 2, 4)
    V_all = np.ascontiguousarray(V_all).astype(fp8)

    # weights: [P, CH, C(out)] with c_in = ci*128 + p
    Wq8 = np.ascontiguousarray(
        (Wq.T * SCALE).reshape(CH, P, C).transpose(1, 0, 2)).astype(fp8)
    Wo8 = np.ascontiguousarray(
        Wo.T.reshape(CH, P, C).transpose(1, 0, 2)).astype(fp8)
    Wkv8 = np.ascontiguousarray(np.stack([
        Wk.T.reshape(CH, P, C).transpose(1, 0, 2),
        Wv.T.reshape(CH, P, C).transpose(1, 0, 2)])).astype(fp8)
    bqs = np.ascontiguousarray((bq * SCALE).reshape(CH, P).T)  # [P, CH]
    bvb = np.ascontiguousarray(np.tile(bv, (M, 1)))
    bob = np.ascontiguousarray(np.tile(bo, (M, 1)))
    # all-batch x, transposed (for the key/value output slice projection)
    xall = x.reshape(B * T, C)
    xTall = np.ascontiguousarray(
        xall.reshape(B * T, CH, P).transpose(2, 1, 0)).astype(bf16)

    in_maps = []
    for c in range(NCORES):
        xc = x[c * BPC:(c + 1) * BPC].reshape(M, C)
        xT = np.ascontiguousarray(xc.reshape(M, CH, P).transpose(2, 1, 0))
        csl = slice(c * P, (c + 1) * P)
        Wslc = np.concatenate([Wk.T[:, csl], Wv.T[:, csl]], axis=1)
        Wslc = np.ascontiguousarray(
            Wslc.reshape(CH, P, 2 * P).transpose(1, 0, 2)).astype(bf16)
        in_maps.append({
            "xT8": xT.astype(fp8),
            "xTall": xTall,
            "Wq8": Wq8.reshape(P, CH * C),
            "Wo8": Wo8.reshape(P, CH * C),
            "Wkv8": Wkv8.reshape(2, P, CH * C),
            "Wsl": Wslc.reshape(P, CH * 2 * P),
            "bvsl": np.ascontiguousarray(np.tile(bv[csl], (B * T, 1))),
            "KT": np.ascontiguousarray(KT_all[c * BPC:(c + 1) * BPC]).reshape(
                BPC, NW, P, CH * 512),
            "Vd": np.ascontiguousarray(V_all[c * BPC:(c + 1) * BPC]).reshape(
                BPC, L // (P * NV), P, NV * C),
            "bqs": bqs, "bvb": bvb, "bob": bob,
        })
    return in_maps


def kernel(x, kv_cache, Wq, bq, Wk, Wv, bv, Wo, bo, _trace=False, _tmpdir=None):
    from concourse.bass_utils import run_bass_kernel_spmd

    _ensure_ntff_hook()
    if "nc" not in _CACHE:
        _CACHE["nc"] = _build()
    nc = _CACHE["nc"]

    in_maps = _prep_host(x, kv_cache, Wq, bq, Wk, Wv, bv, Wo, bo)
    res = run_bass_kernel_spmd(
        nc, in_maps, core_ids=list(range(NCORES)),
        trace=_trace, tmpdir=_tmpdir,
    )
    out = np.empty((B, T, C), np.float32)
    key_o = np.empty((B, T, C), np.float32)
    val_o = np.empty((B, T, C), np.float32)
    for c in range(NCORES):
        r = res.results[c]
        sl = slice(c * BPC, (c + 1) * BPC)
        out[sl] = r["out"].reshape(BPC, T, C)
        csl = slice(c * P, (c + 1) * P)
        key_o[:, :, csl] = r["key_sl"].reshape(B, T, P)
        val_o[:, :, csl] = r["val_sl"].reshape(B, T, P)
    kernel._last_exec_time_ns = res.exec_time_ns
    kernel._last_results = res
    return (out, key_o, val_o)


# revision 39
# speedup vs baseline: 1.0986x; 1.0215x over previous
"""Trainium2 Bass kernel for cached multi-head self-attention decode step.

Problem (hardcoded):
  B=16, T=8, C=1024, n_head=16, head_dim=64, Lcache=4096, layer index 1.
  reference:
    q = x@Wq.T + bq ; key = x@Wk.T ; value = x@Wv.T + bv
    K = concat(kv_cache[:,1,0], key) ; V = concat(kv_cache[:,1,1], value)
    out = softmax((q*s)(K*s)^T) @ V @ Wo.T + bo      (s = hd**-0.25)
    returns (out, key, value)

Sharding: data-parallel over batch. 8 cores x 2 batches each. No collectives.

v4 design:
  - fp8 DoubleRow matmuls (contract 256/instr, measured ~225ns warm at
    N=512 - 2x bf16 per contract) for scores, S@V, q-proj, out-proj.
  - all transposes via matmul with identity rhs.
  - ONE ordered sync-queue DMA stream; PE program order aligned with DMA
    completion order so the PE never head-of-line blocks:
      x,Wq | KT0 | V0s0 | K1w01 | Wk | V0s1 | K1w23 | V0s2 | K1w45 |
      V0s3 | K1w67 | Wv | V1 | Wo
    PE: warmup, q, scores0+T0, sv0(c0-3), s1w01+T1, kproj+kT+newkey0,
      sv0(c4-7), s1w23, sv0(c8-11), s1w45, sv0(c12-15), s1w67, vproj,
      svfinal0, On0, newkey1, gather0, sv1, svfinal1, On1, gather1+outproj.
  - PE kept dense so the HAM clock gate stays at 8/8 (cold MMs are 2x).
"""

import sys
import types

import numpy as np
import ml_dtypes

# ---- hardcoded problem geometry ----
B, T, C = 16, 8, 1024
H, HD = 16, 64
L = 4096            # cached length
LT = L + T          # total keys
NCORES = 8
BPC = B // NCORES   # batches per core = 2
M = BPC * T         # queries per core = 16
P = 128
CH = C // P         # 8 c-chunks
NW = L // 512       # 8 score windows of 512
NV = 8              # V l-chunks (128 rows) per DMA tile (1MB transfers)
NDR = L // 256      # 16 DoubleRow l-pair chunks per batch
NWARM = 14          # PE warmup matmuls of N=512 (HAM un-throttle + stay busy
#                     until the first weights land ~13us in)
SCALE = float(HD) ** -0.5  # folded into Wq/bq on host

# softmax logit shift: exp(s + ESHIFT); cancels in normalization, keeps the
# fp8 S@V weights well inside e4m3 range.
ESHIFT = -2.0

_CACHE = {}


def _ensure_ntff_hook():
    """run_bass_kernel_spmd(trace=True) under axon needs antenv.axon_hooks;
    shim it from the boot module if the image's antenv lacks it."""
    try:
        import antenv.axon_hooks  # noqa: F401
        return
    except ImportError:
        pass
    try:
        import trn_agent_boot.trn_boot as tb
        hook = tb._ntff_profile_via_ctypes("/opt/axon/libaxon_pjrt.so")
    except Exception:
        hook = None
    mod = types.ModuleType("antenv.axon_hooks")
    mod.get_axon_ntff_profile_hook = lambda: hook
    mod.set_axon_ntff_profile_hook = lambda h: None
    sys.modules["antenv.axon_hooks"] = mod


def _build():
    import concourse.bacc as bacc
    import concourse.mybir as mybir
    import concourse.tile as tile
    from concourse.masks import make_identity

    f32 = mybir.dt.float32
    bf16 = mybir.dt.bfloat16
    fp8 = mybir.dt.float8e4
    DR = mybir.MatmulPerfMode.DoubleRow

    nc = bacc.Bacc(None, target_bir_lowering=False)

    # ---- dram I/O (all host-repacked for contiguous loads) ----
    xT8 = nc.dram_tensor("xT8", [P, CH, M], fp8, kind="ExternalInput")
    Wq8 = nc.dram_tensor("Wq8", [P, CH * C], fp8, kind="ExternalInput")
    Wo8 = nc.dram_tensor("Wo8", [P, CH * C], fp8, kind="ExternalInput")
    Wkv8 = nc.dram_tensor("Wkv8", [2, P, CH * C], fp8, kind="ExternalInput")
    xTall = nc.dram_tensor("xTall", [P, CH, B * T], bf16,
                           kind="ExternalInput")
    Wsl = nc.dram_tensor("Wsl", [P, CH * 2 * P], bf16, kind="ExternalInput")
    bvsl = nc.dram_tensor("bvsl", [B * T, P], f32, kind="ExternalInput")
    KT = nc.dram_tensor("KT", [BPC, NW, P, CH * 512], fp8, kind="ExternalInput")
    Vd = nc.dram_tensor("Vd", [BPC, L // (P * NV), P, NV * C], fp8,
                        kind="ExternalInput")
    bqs = nc.dram_tensor("bqs", [P, CH], f32, kind="ExternalInput")
    bvb = nc.dram_tensor("bvb", [M, C], f32, kind="ExternalInput")
    bob = nc.dram_tensor("bob", [M, C], f32, kind="ExternalInput")
    out_d = nc.dram_tensor("out", [M, C], f32, kind="ExternalOutput")
    ksl_d = nc.dram_tensor("key_sl", [B * T, P], f32, kind="ExternalOutput")
    vsl_d = nc.dram_tensor("val_sl", [B * T, P], f32, kind="ExternalOutput")

    AF = mybir.ActivationFunctionType
    AX = mybir.AxisListType
    OP = mybir.AluOpType

    with tile.TileContext(nc) as tc:
        with (
            tc.tile_pool(name="const", bufs=1) as cpool,
            tc.tile_pool(name="kt", bufs=8) as ktpool,
            tc.tile_pool(name="v", bufs=8) as vpool,
            tc.tile_pool(name="nat", bufs=2) as natpool,
            tc.tile_pool(name="wchunk", bufs=2 * NW + 2) as wtpool,
            tc.tile_pool(name="big", bufs=1) as big,
            tc.tile_pool(name="ps", bufs=1, space="PSUM") as pp,
        ):
            # ---------------- DMA issue order == consumption order --------
            xT8_sb = cpool.tile([P, CH, M], fp8, tag="xT8", name="xT8")
            nc.sync.dma_start(xT8_sb[:], xT8[:])
            wq_sb = cpool.tile([P, CH, C], fp8, tag="wq", name="wq")
            nc.sync.dma_start(wq_sb[:], Wq8[:])
            kts = {}

            def kt_dma(b, wp):
                # one 1MB transfer = two score windows (2w, 2w+1)
                kts[(b, wp)] = ktpool.tile([P, 2, CH, 512], fp8, tag="kt",
                                           name="kt")
                nc.sync.dma_start(
                    kts[(b, wp)][:],
                    KT[b, 2 * wp:2 * wp + 2].rearrange("i p r -> p i r"))

            vts = {}

            def v_dma(b, s):
                vts[(b, s)] = vpool.tile([P, NV, C], fp8, tag="v", name="v")
                nc.sync.dma_start(vts[(b, s)][:], Vd[b, s])

            wkv_sb = cpool.tile([P, 2, CH, C], fp8, tag="wkv", name="wkv")
            xall_sb = cpool.tile([P, CH, B * T], bf16, tag="xall", name="xall")
            wsl_sb = cpool.tile([P, CH, 2 * P], bf16, tag="wsl", name="wsl")
            bvsl_sb = cpool.tile([B * T, P], f32, tag="bvsl", name="bvsl")

            for wp in range(NW // 2):
                kt_dma(0, wp)
            v_dma(0, 0)
            kt_dma(1, 0)
            nc.sync.dma_start(wkv_sb[:, 0], Wkv8[0])   # Wk fp8
            v_dma(0, 1)
            kt_dma(1, 1)
            nc.sync.dma_start(xall_sb[:], xTall[:])
            nc.sync.dma_start(wsl_sb[:], Wsl[:])
            nc.sync.dma_start(bvsl_sb[:], bvsl[:])
            v_dma(0, 2)
            kt_dma(1, 2)
            v_dma(0, 3)
            kt_dma(1, 3)
            nc.sync.dma_start(wkv_sb[:, 1], Wkv8[1])   # Wv fp8
            for s in range(4):
                v_dma(1, s)
            wo_sb = cpool.tile([P, CH, C], fp8, tag="wo", name="wo")
            nc.sync.dma_start(wo_sb[:], Wo8[:])

            # ---- constants / memsets (gpsimd memsets BEFORE its slow
            # SWDGE dma emissions so the PE warmup input is ready early) ----
            warm_in = cpool.tile([P, 512], bf16, tag="warm_in", name="warm_in")
            nc.gpsimd.memset(warm_in[:], 0.25)
            eshift = cpool.tile([P, 1], f32, tag="eshift", name="eshift")
            nc.gpsimd.memset(eshift[:], ESHIFT)
            Qb = {}
            for b in range(BPC):
                Qb[b] = big.tile([P, CH, P], fp8, tag=f"Qbig{b}",
                                 name=f"Qbig{b}")
                nc.gpsimd.memset(Qb[b][:], 0.0)
            vpad = big.tile([P, C], fp8, tag="vpad", name="vpad")
            nc.gpsimd.memset(vpad[:], 0.0)
            wn_pad, wt32 = {}, {}
            for b in range(BPC):
                wn_pad[b] = big.tile([P, M], fp8, tag=f"wn_pad{b}",
                                     name=f"wn_pad{b}")
                nc.gpsimd.memset(wn_pad[b][:], 0.0)
                wt32[b] = big.tile([P, P], fp8, tag=f"wt32_{b}",
                                   name=f"wt32_{b}")
                nc.gpsimd.memset(wt32[b][:], 0.0)
            bqs_sb = cpool.tile([P, CH], f32, tag="bqs", name="bqs")
            nc.gpsimd.dma_start(bqs_sb[:], bqs[:])
            bvb_sb = cpool.tile([M, C], bf16, tag="bvb", name="bvb")
            nc.gpsimd.dma_start(bvb_sb[:], bvb[:])
            bob_sb = cpool.tile([M, C], bf16, tag="bob", name="bob")
            nc.gpsimd.dma_start(bob_sb[:], bob[:])

            ident = cpool.tile([P, P], f32, tag="ident", name="ident")
            make_identity(nc, ident)
            ident_b = cpool.tile([P, P], bf16, tag="ident_b", name="ident_b")
            nc.vector.tensor_copy(out=ident_b[:], in_=ident[:])
            ident_8 = cpool.tile([P, P], fp8, tag="ident_8", name="ident_8")
            nc.vector.tensor_copy(out=ident_8[:], in_=ident[:])

            # ---- PE warmup / filler: keep the HAM clock gate at 8/8 (cold
            # matmuls run at 1.2 vs 2.4 GHz). Each call allocates from the
            # t-tag ring so WAR deps are tracked.
            _wk = [0]

            def warm_fill(n, nfree=64):
                wm = pp.tile([P, 512], f32, tag=f"t{_wk[0] % 2}", name="warm")
                _wk[0] += 1
                for _ in range(n):
                    nc.tensor.matmul(wm[:, 0:nfree], warm_in[:, 0:P],
                                     warm_in[:, 0:nfree],
                                     start=True, stop=True)

            warm_fill(NWARM, nfree=512)

            # ---------------- per-batch state ------------------------------
            W_s, sums, rsum, ops_b, On = {}, {}, {}, {}, {}
            for b in range(BPC):
                W_s[b] = big.tile([P, LT], fp8, tag=f"W{b}", name=f"W{b}")
                sums[b] = big.tile([P, NW + 1], f32, tag=f"sums{b}",
                                   name=f"sums{b}")
                On[b] = big.tile([P, C], bf16, tag=f"On{b}", name=f"On{b}")
            wts = {0: {}, 1: {}}  # (b -> chunk -> wt tile)

            # ---------------- Phase A: q projection (fp8 DR) --------------
            q_bf = big.tile([M, C], bf16, tag="q_bf", name="q_bf")
            for j in range(2):
                qps = pp.tile([M, 512], f32, tag=f"s{j}", name=f"qps{j}")
                for ci in range(0, CH, 2):
                    nc.tensor.matmul(
                        qps[:], xT8_sb[:, ci:ci + 2, :],
                        wq_sb[:, ci:ci + 2, j * 512:(j + 1) * 512],
                        start=(ci == 0), stop=(ci == CH - 2), perf_mode=DR,
                    )
                nc.scalar.copy(q_bf[:, j * 512:(j + 1) * 512], qps[:])
            for co in range(CH):
                tpq = pp.tile([P, P], f32, tag=f"t{co % 2}", name="tpq")
                nc.tensor.matmul(
                    tpq[:, 0:M], q_bf[:, co * P:(co + 1) * P],
                    ident_b[0:M, 0:M], start=True, stop=True,
                )
                for b in range(BPC):
                    for j in range(2):
                        rows = slice(64 * j, 64 * (j + 1))
                        nc.scalar.activation(
                            Qb[b][rows, co, 16 * co + 8 * j:16 * co + 8 * j + 8],
                            tpq[rows, b * T:b * T + T],
                            AF.Identity, bias=bqs_sb[rows, co:co + 1],
                        )

            def w_transpose(b, t):
                """wt pair tile for DR l-chunk t of batch b (2 matmul-T)."""
                wt = wtpool.tile([P, 2, P], fp8, tag="wt", name="wt")
                for i in range(2):
                    tpw = pp.tile([P, P], f32, tag=f"t{(2 * t + i) % 2}",
                                  name="tpw")
                    nc.tensor.matmul(
                        tpw[:], W_s[b][:, (2 * t + i) * P:(2 * t + i + 1) * P],
                        ident_8[:], start=True, stop=True,
                    )
                    nc.vector.tensor_copy(out=wt[:, i, :], in_=tpw[:])
                wts[b][t] = wt

            def scores_window(b, lw, transpose=True):
                kt = kts[(b, lw // 2)][:, lw % 2]
                sp = pp.tile([P, 512], f32, tag=f"s{lw % 2}", name="sp")
                for ci in range(0, CH, 2):
                    nc.tensor.matmul(
                        sp[:], Qb[b][:, ci:ci + 2, :], kt[:, ci:ci + 2, :],
                        start=(ci == 0), stop=(ci == CH - 2), perf_mode=DR,
                    )
                nc.scalar.activation(
                    W_s[b][:, lw * 512:(lw + 1) * 512], sp[:], AF.Exp,
                    bias=eshift[:, 0:1],
                    accum_out=sums[b][:, lw:lw + 1],
                )
                if transpose and lw > 0:
                    w_transpose(b, 2 * (lw - 1))
                    w_transpose(b, 2 * (lw - 1) + 1)

            def scores_tail(b):
                w_transpose(b, 2 * (NW - 1))
                w_transpose(b, 2 * (NW - 1) + 1)

            def sv_chunks(b, t0, t1):
                """pure DR S@V over l-pair chunks [t0, t1)."""
                if t0 == 0:
                    ops_b[b] = [pp.tile([P, 512], f32, tag=f"o{2 * b + j}",
                                        name=f"sv{b}{j}") for j in range(2)]
                ops = ops_b[b]
                for t_ in range(t0, t1):
                    vt = vts[(b, t_ // 4)]
                    tt = t_ % 4
                    for j in range(2):
                        nc.tensor.matmul(
                            ops[j][:], wts[b][t_][:],
                            vt[:, 2 * tt:2 * tt + 2, j * 512:(j + 1) * 512],
                            start=(t_ == 0), stop=False, perf_mode=DR,
                        )
                    del wts[b][t_]

            def scores_newkey_a(b):
                # new-key scores + normalization sums + wn_pad staging; the
                # DVE tail here runs under whatever PE work follows.
                spn = pp.tile([P, 512], f32, tag=f"s{b % 2}", name="spn")
                for ci in range(0, CH, 2):
                    nc.tensor.matmul(
                        spn[:, 0:T], Qb[b][:, ci:ci + 2, :],
                        kT[:, ci:ci + 2, b * T:(b + 1) * T],
                        start=(ci == 0), stop=(ci == CH - 2), perf_mode=DR,
                    )
                nc.scalar.activation(
                    W_s[b][:, L:LT], spn[:, 0:T], AF.Exp,
                    bias=eshift[:, 0:1], accum_out=sums[b][:, NW:NW + 1],
                )
                rs = big.tile([P, 1], f32, tag=f"rs{b}", name=f"rs{b}")
                nc.vector.tensor_reduce(out=rs[:], in_=sums[b][:],
                                        axis=AX.X, op=OP.add)
                rsum[b] = big.tile([P, 1], f32, tag=f"rsum{b}", name=f"rsum{b}")
                nc.vector.reciprocal(rsum[b][:], rs[:])
                nc.vector.tensor_copy(out=wn_pad[b][:, b * T:(b + 1) * T],
                                      in_=W_s[b][:, L:LT])

            def scores_newkey_b(b):
                tpn = pp.tile([P, P], f32, tag=f"t{b % 2}", name="tpn")
                nc.tensor.matmul(tpn[0:M, :], wn_pad[b][:], ident_8[:],
                                 start=True, stop=True)
                nc.vector.tensor_copy(out=wt32[b][0:M, :], in_=tpn[0:M, :])

            def sv_final(b):
                ops = ops_b[b]
                for j in range(2):
                    nc.tensor.matmul(
                        ops[j][:], wt32[b][:], vpad[:, j * 512:(j + 1) * 512],
                        start=False, stop=True,
                    )
                    nc.scalar.activation(
                        On[b][:, j * 512:(j + 1) * 512], ops[j][:], AF.Copy,
                        scale=rsum[b][:],
                    )

            wvT = big.tile([P, CH, M], fp8, tag="wvT", name="wvT")

            def gather_ci(b, ci):
                tp = pp.tile([P, P], f32, tag=f"t{ci % 2}", name="tpg")
                nc.tensor.matmul(tp[:], On[b][:, ci * P:(ci + 1) * P],
                                 ident_b[:], start=True, stop=True)
                nc.vector.tensor_copy(
                    out=wvT[0:64, ci, b * T:(b + 1) * T],
                    in_=tp[0:64, 16 * ci:16 * ci + 8])
                nc.vector.tensor_copy(
                    out=wvT[64:P, ci, b * T:(b + 1) * T],
                    in_=tp[64:P, 16 * ci + 8:16 * ci + 16])

            # ---- k/v projections ----
            # attention-grade (fp8 DR) k/v for the core's own 2 batches;
            # output-grade (bf16) key/value c-slices for ALL batches.
            k_bf = big.tile([M, C], bf16, tag="k_bf", name="k_bf")
            kT = big.tile([P, CH, M], fp8, tag="kT", name="kT")
            ksl = natpool.tile([B * T, P], f32, tag="nat", name="ksl")
            vsl = natpool.tile([B * T, P], f32, tag="nat", name="vsl")

            def kv_proj(half):
                for j in range(2):
                    ps = pp.tile([M, 512], f32, tag=f"o{2 + j}",
                                 name=f"kv{half}{j}")
                    for ci in range(0, CH, 2):
                        nc.tensor.matmul(
                            ps[:], xT8_sb[:, ci:ci + 2, :],
                            wkv_sb[:, half, ci:ci + 2, j * 512:(j + 1) * 512],
                            start=(ci == 0), stop=(ci == CH - 2), perf_mode=DR,
                        )
                    sl = slice(j * 512, (j + 1) * 512)
                    if half == 0:
                        nc.vector.tensor_copy(out=k_bf[:, sl], in_=ps[:])
                    else:
                        nc.vector.tensor_add(out=vpad[0:M, sl], in0=ps[:],
                                             in1=bvb_sb[:, sl])

            def slice_proj():
                # key/value output c-slice for ALL 16 batches (bf16 grade)
                ps = pp.tile([B * T, 2 * P], f32, tag="o2", name="psl")
                for ci in range(CH):
                    nc.tensor.matmul(
                        ps[:], xall_sb[:, ci, :], wsl_sb[:, ci, :],
                        start=(ci == 0), stop=(ci == CH - 1),
                    )
                nc.scalar.copy(ksl[:], ps[:, 0:P])
                nc.vector.tensor_add(out=vsl[:], in0=ps[:, P:2 * P],
                                     in1=bvsl_sb[:])
                nc.scalar.dma_start(ksl_d[:], ksl[:])
                nc.scalar.dma_start(vsl_d[:], vsl[:])

            def kT_piece():
                for ci in range(CH):
                    tp = pp.tile([P, P], f32, tag=f"t{ci % 2}", name="tpk")
                    nc.tensor.matmul(tp[:, 0:M], k_bf[:, ci * P:(ci + 1) * P],
                                     ident_b[0:M, 0:M], start=True, stop=True)
                    nc.vector.tensor_copy(out=kT[:, ci, :], in_=tp[:, 0:M])

            # ============ main schedule (consumption-order aligned) ========
            for w in range(NW):                      # scores(0) + T(0)
                scores_window(0, w)
            scores_tail(0)
            sv_chunks(0, 0, 4)                       # <- V0s0
            scores_window(1, 0)                      # <- K1w01
            scores_window(1, 1)
            kv_proj(0)                               # <- Wk
            kT_piece()
            scores_newkey_a(0)
            sv_chunks(0, 4, 8)                       # <- V0s1
            scores_newkey_b(0)
            scores_window(1, 2)                      # <- K1w23
            scores_window(1, 3)
            slice_proj()                             # <- Wsl/xTall
            sv_chunks(0, 8, 12)                      # <- V0s2
            scores_window(1, 4)                      # <- K1w45
            scores_window(1, 5)
            sv_chunks(0, 12, 16)                     # <- V0s3
            scores_window(1, 6)                      # <- K1w67
            scores_window(1, 7)
            scores_tail(1)
            scores_newkey_a(1)
            kv_proj(1)                               # <- Wv
            scores_newkey_b(1)
            sv_final(0)
            for ci in range(CH):
                gather_ci(0, ci)
            sv_chunks(1, 0, 4)                       # <- V1s0
            sv_chunks(1, 4, 8)
            sv_chunks(1, 8, 12)
            sv_chunks(1, 12, 16)
            sv_final(1)

            # gather(1) + out projection (DR), 1-pair chase     <- Wo
            ps_fin = [pp.tile([M, 512], f32, tag=f"s{j}", name=f"fin{j}")
                      for j in range(2)]
            fin = natpool.tile([M, C], f32, tag="nat", name="fin")
            gather_ci(1, 0)
            gather_ci(1, 1)
            for pair in range(CH // 2):
                if pair < 3:
                    gather_ci(1, 2 * pair + 2)
                    gather_ci(1, 2 * pair + 3)
                ci = 2 * pair
                for j in range(2):
                    nc.tensor.matmul(
                        ps_fin[j][:], wvT[:, ci:ci + 2, :],
                        wo_sb[:, ci:ci + 2, j * 512:(j + 1) * 512],
                        start=(pair == 0), stop=(pair == CH // 2 - 1),
                        perf_mode=DR,
                    )
            for j in range(2):
                sl = slice(j * 512, (j + 1) * 512)
                nc.vector.tensor_add(out=fin[:, sl], in0=ps_fin[j][:],
                                     in1=bob_sb[:, sl])
                nc.scalar.dma_start(out_d[:, sl], fin[:, sl])

    nc.compile()
    return nc


def _prep_host(x, kv_cache, Wq, bq, Wk, Wv, bv, Wo, bo):
    fp8 = ml_dtypes.float8_e4m3
    bf16 = ml_dtypes.bfloat16
    f32 = np.float32
    x = np.asarray(x, f32)
    kv = np.asarray(kv_cache)
    Wq = np.asarray(Wq, f32); bq = np.asarray(bq, f32)
    Wk = np.asarray(Wk, f32); Wv = np.asarray(Wv, f32); bv = np.asarray(bv, f32)
    Wo = np.asarray(Wo, f32); bo = np.asarray(bo, f32)

    # K-cache / V-cache repacked so every device DMA is a fully contiguous
    # [128 x >=4KB] transfer:
    #   KT[b, w, p, ci*512 + j] = K[b, w*512 + j, ci*128 + p]
    #   Vd[b, s, p, tt*C + c]   = V[b, (s*NV + tt)*128 + p, c]
    KT_all = np.asarray(kv[:, 1, 0], f32).transpose(0, 2, 1).reshape(
        B, CH, P, NW, 512).transpose(0, 3, 2, 1, 4)
    KT_all = np.ascontiguousarray(KT_all).astype(fp8)
    V_all = np.asarray(kv[:, 1, 1], f32).reshape(
        B, L // (P * NV), NV, P, C).transpose(0, 1, 3, 2, 4)
    V_all = np.ascontiguousarray(V_all).astype(fp8)

    # weights: [P, CH, C(out)] with c_in = ci*128 + p
    Wq8 = np.ascontiguousarray(
        (Wq.T * SCALE).reshape(CH, P, C).transpose(1, 0, 2)).astype(fp8)
    Wo8 = np.ascontiguousarray(
        Wo.T.reshape(CH, P, C).transpose(1, 0, 2)).astype(fp8)
    Wkv8 = np.ascontiguousarray(np.stack([
        Wk.T.reshape(CH, P, C).transpose(1, 0, 2),
        Wv.T.reshape(CH, P, C).transpose(1, 0, 2)])).astype(fp8)
    bqs = np.ascontiguousarray((bq * SCALE).reshape(CH, P).T)  # [P, CH]
    bvb = np.ascontiguousarray(np.tile(bv, (M, 1)))
    bob = np.ascontiguousarray(np.tile(bo, (M, 1)))
    # all-batch x, transposed (for the key/value output slice projection)
    xall = x.reshape(B * T, C)
    xTall = np.ascontiguousarray(
        xall.reshape(B * T, CH, P).transpose(2, 1, 0)).astype(bf16)

    in_maps = []
    for c in range(NCORES):
        xc = x[c * BPC:(c + 1) * BPC].reshape(M, C)
        xT = np.ascontiguousarray(xc.reshape(M, CH, P).transpose(2, 1, 0))
        csl = slice(c * P, (c + 1) * P)
        Wslc = np.concatenate([Wk.T[:, csl], Wv.T[:, csl]], axis=1)
        Wslc = np.ascontiguousarray(
            Wslc.reshape(CH, P, 2 * P).transpose(1, 0, 2)).astype(bf16)
        in_maps.append({
            "xT8": xT.astype(fp8),
            "xTall": xTall,
            "Wq8": Wq8.reshape(P, CH * C),
            "Wo8": Wo8.reshape(P, CH * C),
            "Wkv8": Wkv8.reshape(2, P, CH * C),
            "Wsl": Wslc.reshape(P, CH * 2 * P),
            "bvsl": np.ascontiguousarray(np.tile(bv[csl], (B * T, 1))),
            "KT": np.ascontiguousarray(KT_all[c * BPC:(c + 1) * BPC]).reshape(
                BPC, NW, P, CH * 512),
            "Vd": np.ascontiguousarray(V_all[c * BPC:(c + 1) * BPC]).reshape(
                BPC, L // (P * NV), P, NV * C),
            "bqs": bqs, "bvb": bvb, "bob": bob,
        })
    return in_maps


def kernel(x, kv_cache, Wq, bq, Wk, Wv, bv, Wo, bo, _trace=False, _tmpdir=None):
    from concourse.bass_utils import run_bass_kernel_spmd

    _ensure_ntff_hook()
    if "nc" not in _CACHE:
        _CACHE["nc"] = _build()
    nc = _CACHE["nc"]

    in_maps = _prep_host(x, kv_cache, Wq, bq, Wk, Wv, bv, Wo, bo)
    res = run_bass_kernel_spmd(
        nc, in_maps, core_ids=list(range(NCORES)),
        trace=_trace, tmpdir=_tmpdir,
    )
    out = np.empty((B, T, C), np.float32)
    key_o = np.empty((B, T, C), np.float32)
    val_o = np.empty((B, T, C), np.float32)
    for c in range(NCORES):
        r = res.results[c]
        sl = slice(c * BPC, (c + 1) * BPC)
        out[sl] = r["out"].reshape(BPC, T, C)
        csl = slice(c * P, (c + 1) * P)
        key_o[:, :, csl] = r["key_sl"].reshape(B, T, P)
        val_o[:, :, csl] = r["val_sl"].reshape(B, T, P)
    kernel._last_exec_time_ns = res.exec_time_ns
    kernel._last_results = res
    return (out, key_o, val_o)


# revision 43
# speedup vs baseline: 1.1133x; 1.0134x over previous
"""Trainium2 Bass kernel for cached multi-head self-attention decode step.

Problem (hardcoded):
  B=16, T=8, C=1024, n_head=16, head_dim=64, Lcache=4096, layer index 1.
  reference:
    q = x@Wq.T + bq ; key = x@Wk.T ; value = x@Wv.T + bv
    K = concat(kv_cache[:,1,0], key) ; V = concat(kv_cache[:,1,1], value)
    out = softmax((q*s)(K*s)^T) @ V @ Wo.T + bo      (s = hd**-0.25)
    returns (out, key, value)

Sharding: data-parallel over batch. 8 cores x 2 batches each. No collectives.

v4 design:
  - fp8 DoubleRow matmuls (contract 256/instr, measured ~225ns warm at
    N=512 - 2x bf16 per contract) for scores, S@V, q-proj, out-proj.
  - all transposes via matmul with identity rhs.
  - ONE ordered sync-queue DMA stream; PE program order aligned with DMA
    completion order so the PE never head-of-line blocks:
      x,Wq | KT0 | V0s0 | K1w01 | Wk | V0s1 | K1w23 | V0s2 | K1w45 |
      V0s3 | K1w67 | Wv | V1 | Wo
    PE: warmup, q, scores0+T0, sv0(c0-3), s1w01+T1, kproj+kT+newkey0,
      sv0(c4-7), s1w23, sv0(c8-11), s1w45, sv0(c12-15), s1w67, vproj,
      svfinal0, On0, newkey1, gather0, sv1, svfinal1, On1, gather1+outproj.
  - PE kept dense so the HAM clock gate stays at 8/8 (cold MMs are 2x).
"""

import sys
import types

import numpy as np
import ml_dtypes

# ---- hardcoded problem geometry ----
B, T, C = 16, 8, 1024
H, HD = 16, 64
L = 4096            # cached length
LT = L + T          # total keys
NCORES = 8
BPC = B // NCORES   # batches per core = 2
M = BPC * T         # queries per core = 16
P = 128
CH = C // P         # 8 c-chunks
NW = L // 512       # 8 score windows of 512
NV = 8              # V l-chunks (128 rows) per DMA tile (1MB transfers)
NDR = L // 256      # 16 DoubleRow l-pair chunks per batch
NWARM = 14          # PE warmup matmuls of N=512 (HAM un-throttle + stay busy
#                     until the first weights land ~13us in)
SCALE = float(HD) ** -0.5  # folded into Wq/bq on host

# softmax logit shift: exp(s + ESHIFT); cancels in normalization, keeps the
# fp8 S@V weights well inside e4m3 range.
ESHIFT = -2.0

_CACHE = {}


def _ensure_ntff_hook():
    """run_bass_kernel_spmd(trace=True) under axon needs antenv.axon_hooks;
    shim it from the boot module if the image's antenv lacks it."""
    try:
        import antenv.axon_hooks  # noqa: F401
        return
    except ImportError:
        pass
    try:
        import trn_agent_boot.trn_boot as tb
        hook = tb._ntff_profile_via_ctypes("/opt/axon/libaxon_pjrt.so")
    except Exception:
        hook = None
    mod = types.ModuleType("antenv.axon_hooks")
    mod.get_axon_ntff_profile_hook = lambda: hook
    mod.set_axon_ntff_profile_hook = lambda h: None
    sys.modules["antenv.axon_hooks"] = mod


def _build():
    import concourse.bacc as bacc
    import concourse.mybir as mybir
    import concourse.tile as tile
    from concourse.masks import make_identity

    f32 = mybir.dt.float32
    bf16 = mybir.dt.bfloat16
    fp8 = mybir.dt.float8e4
    DR = mybir.MatmulPerfMode.DoubleRow

    nc = bacc.Bacc(None, target_bir_lowering=False)

    # ---- dram I/O (all host-repacked for contiguous loads) ----
    xT8 = nc.dram_tensor("xT8", [P, CH, M], fp8, kind="ExternalInput")
    Wq8 = nc.dram_tensor("Wq8", [P, CH * C], fp8, kind="ExternalInput")
    Wo8 = nc.dram_tensor("Wo8", [P, CH * C], fp8, kind="ExternalInput")
    Wkv8 = nc.dram_tensor("Wkv8", [2, P, CH * C], fp8, kind="ExternalInput")
    xTall = nc.dram_tensor("xTall", [P, CH, B * T], bf16,
                           kind="ExternalInput")
    Wsl = nc.dram_tensor("Wsl", [P, CH * 2 * P], bf16, kind="ExternalInput")
    bvsl = nc.dram_tensor("bvsl", [B * T, P], f32, kind="ExternalInput")
    KT = nc.dram_tensor("KT", [BPC, NW // 2, P, 2 * CH * 512], fp8,
                        kind="ExternalInput")
    Vd = nc.dram_tensor("Vd", [BPC, L // (P * NV), P, NV * C], fp8,
                        kind="ExternalInput")
    bqs = nc.dram_tensor("bqs", [P, CH], f32, kind="ExternalInput")
    bvb = nc.dram_tensor("bvb", [M, C], f32, kind="ExternalInput")
    bob = nc.dram_tensor("bob", [M, C], f32, kind="ExternalInput")
    out_d = nc.dram_tensor("out", [M, C], f32, kind="ExternalOutput")
    ksl_d = nc.dram_tensor("key_sl", [B * T, P], f32, kind="ExternalOutput")
    vsl_d = nc.dram_tensor("val_sl", [B * T, P], f32, kind="ExternalOutput")

    AF = mybir.ActivationFunctionType
    AX = mybir.AxisListType
    OP = mybir.AluOpType

    with tile.TileContext(nc) as tc:
        with (
            tc.tile_pool(name="const", bufs=1) as cpool,
            tc.tile_pool(name="kt", bufs=8) as ktpool,
            tc.tile_pool(name="v", bufs=8) as vpool,
            tc.tile_pool(name="nat", bufs=2) as natpool,
            tc.tile_pool(name="wchunk", bufs=2 * NW + 2) as wtpool,
            tc.tile_pool(name="big", bufs=1) as big,
            tc.tile_pool(name="ps", bufs=1, space="PSUM") as pp,
        ):
            # ---------------- DMA issue order == consumption order --------
            xT8_sb = cpool.tile([P, CH, M], fp8, tag="xT8", name="xT8")
            nc.sync.dma_start(xT8_sb[:], xT8[:])
            wq_sb = cpool.tile([P, CH, C], fp8, tag="wq", name="wq")
            nc.sync.dma_start(wq_sb[:], Wq8[:])
            kts = {}

            def kt_dma(b, wp):
                # one contiguous 1MB transfer = two score windows (2w, 2w+1)
                kts[(b, wp)] = ktpool.tile([P, 2, CH, 512], fp8, tag="kt",
                                           name="kt")
                nc.sync.dma_start(kts[(b, wp)][:], KT[b, wp])

            vts = {}

            def v_dma(b, s):
                vts[(b, s)] = vpool.tile([P, NV, C], fp8, tag="v", name="v")
                nc.sync.dma_start(vts[(b, s)][:], Vd[b, s])

            wkv_sb = cpool.tile([P, 2, CH, C], fp8, tag="wkv", name="wkv")
            xall_sb = cpool.tile([P, CH, B * T], bf16, tag="xall", name="xall")
            wsl_sb = cpool.tile([P, CH, 2 * P], bf16, tag="wsl", name="wsl")
            bvsl_sb = cpool.tile([B * T, P], f32, tag="bvsl", name="bvsl")

            for wp in range(NW // 2):
                kt_dma(0, wp)
            v_dma(0, 0)
            kt_dma(1, 0)
            nc.sync.dma_start(wkv_sb[:, 0], Wkv8[0])   # Wk fp8
            v_dma(0, 1)
            kt_dma(1, 1)
            nc.sync.dma_start(xall_sb[:], xTall[:])
            nc.sync.dma_start(wsl_sb[:], Wsl[:])
            nc.sync.dma_start(bvsl_sb[:], bvsl[:])
            v_dma(0, 2)
            kt_dma(1, 2)
            v_dma(0, 3)
            kt_dma(1, 3)
            nc.sync.dma_start(wkv_sb[:, 1], Wkv8[1])   # Wv fp8
            for s in range(4):
                v_dma(1, s)
            wo_sb = cpool.tile([P, CH, C], fp8, tag="wo", name="wo")
            nc.sync.dma_start(wo_sb[:], Wo8[:])

            # ---- constants / memsets (gpsimd memsets BEFORE its slow
            # SWDGE dma emissions so the PE warmup input is ready early) ----
            warm_in = cpool.tile([P, 512], bf16, tag="warm_in", name="warm_in")
            nc.gpsimd.memset(warm_in[:], 0.25)
            eshift = cpool.tile([P, 1], f32, tag="eshift", name="eshift")
            nc.gpsimd.memset(eshift[:], ESHIFT)
            Qb = {}
            for b in range(BPC):
                Qb[b] = big.tile([P, CH, P], fp8, tag=f"Qbig{b}",
                                 name=f"Qbig{b}")
                nc.gpsimd.memset(Qb[b][:], 0.0)
            vpad = big.tile([P, C], fp8, tag="vpad", name="vpad")
            nc.gpsimd.memset(vpad[:], 0.0)
            wn_pad, wt32 = {}, {}
            for b in range(BPC):
                wn_pad[b] = big.tile([P, M], fp8, tag=f"wn_pad{b}",
                                     name=f"wn_pad{b}")
                nc.gpsimd.memset(wn_pad[b][:], 0.0)
                wt32[b] = big.tile([P, P], fp8, tag=f"wt32_{b}",
                                   name=f"wt32_{b}")
                nc.gpsimd.memset(wt32[b][:], 0.0)
            bqs_sb = cpool.tile([P, CH], f32, tag="bqs", name="bqs")
            nc.gpsimd.dma_start(bqs_sb[:], bqs[:])
            bvb_sb = cpool.tile([M, C], bf16, tag="bvb", name="bvb")
            nc.gpsimd.dma_start(bvb_sb[:], bvb[:])
            bob_sb = cpool.tile([M, C], bf16, tag="bob", name="bob")
            nc.gpsimd.dma_start(bob_sb[:], bob[:])

            ident = cpool.tile([P, P], f32, tag="ident", name="ident")
            make_identity(nc, ident)
            ident_b = cpool.tile([P, P], bf16, tag="ident_b", name="ident_b")
            nc.vector.tensor_copy(out=ident_b[:], in_=ident[:])
            ident_8 = cpool.tile([P, P], fp8, tag="ident_8", name="ident_8")
            nc.vector.tensor_copy(out=ident_8[:], in_=ident[:])

            # ---- PE warmup / filler: keep the HAM clock gate at 8/8 (cold
            # matmuls run at 1.2 vs 2.4 GHz). Each call allocates from the
            # t-tag ring so WAR deps are tracked.
            _wk = [0]

            def warm_fill(n, nfree=64):
                wm = pp.tile([P, 512], f32, tag=f"t{_wk[0] % 2}", name="warm")
                _wk[0] += 1
                for _ in range(n):
                    nc.tensor.matmul(wm[:, 0:nfree], warm_in[:, 0:P],
                                     warm_in[:, 0:nfree],
                                     start=True, stop=True)

            warm_fill(NWARM, nfree=512)

            # ---------------- per-batch state ------------------------------
            W_s, sums, rsum, ops_b, On = {}, {}, {}, {}, {}
            for b in range(BPC):
                W_s[b] = big.tile([P, LT], fp8, tag=f"W{b}", name=f"W{b}")
                sums[b] = big.tile([P, NW + 1], f32, tag=f"sums{b}",
                                   name=f"sums{b}")
                On[b] = big.tile([P, C], bf16, tag=f"On{b}", name=f"On{b}")
            wts = {0: {}, 1: {}}  # (b -> chunk -> wt tile)

            # ---------------- Phase A: q projection (fp8 DR) --------------
            q_bf = big.tile([M, C], bf16, tag="q_bf", name="q_bf")
            for j in range(2):
                qps = pp.tile([M, 512], f32, tag=f"s{j}", name=f"qps{j}")
                for ci in range(0, CH, 2):
                    nc.tensor.matmul(
                        qps[:], xT8_sb[:, ci:ci + 2, :],
                        wq_sb[:, ci:ci + 2, j * 512:(j + 1) * 512],
                        start=(ci == 0), stop=(ci == CH - 2), perf_mode=DR,
                    )
                nc.scalar.copy(q_bf[:, j * 512:(j + 1) * 512], qps[:])
            for co in range(CH):
                tpq = pp.tile([P, P], f32, tag=f"t{co % 2}", name="tpq")
                nc.tensor.matmul(
                    tpq[:, 0:M], q_bf[:, co * P:(co + 1) * P],
                    ident_b[0:M, 0:M], start=True, stop=True,
                )
                for b in range(BPC):
                    for j in range(2):
                        rows = slice(64 * j, 64 * (j + 1))
                        nc.scalar.activation(
                            Qb[b][rows, co, 16 * co + 8 * j:16 * co + 8 * j + 8],
                            tpq[rows, b * T:b * T + T],
                            AF.Identity, bias=bqs_sb[rows, co:co + 1],
                        )

            def w_transpose(b, t):
                """wt pair tile for DR l-chunk t of batch b (2 matmul-T)."""
                wt = wtpool.tile([P, 2, P], fp8, tag="wt", name="wt")
                for i in range(2):
                    tpw = pp.tile([P, P], f32, tag=f"t{(2 * t + i) % 2}",
                                  name="tpw")
                    nc.tensor.matmul(
                        tpw[:], W_s[b][:, (2 * t + i) * P:(2 * t + i + 1) * P],
                        ident_8[:], start=True, stop=True,
                    )
                    nc.vector.tensor_copy(out=wt[:, i, :], in_=tpw[:])
                wts[b][t] = wt

            def scores_window(b, lw, transpose=True):
                kt = kts[(b, lw // 2)][:, lw % 2]
                sp = pp.tile([P, 512], f32, tag=f"s{lw % 2}", name="sp")
                for ci in range(0, CH, 2):
                    nc.tensor.matmul(
                        sp[:], Qb[b][:, ci:ci + 2, :], kt[:, ci:ci + 2, :],
                        start=(ci == 0), stop=(ci == CH - 2), perf_mode=DR,
                    )
                nc.scalar.activation(
                    W_s[b][:, lw * 512:(lw + 1) * 512], sp[:], AF.Exp,
                    bias=eshift[:, 0:1],
                    accum_out=sums[b][:, lw:lw + 1],
                )
                if transpose and lw > 0:
                    w_transpose(b, 2 * (lw - 1))
                    w_transpose(b, 2 * (lw - 1) + 1)

            def scores_tail(b):
                w_transpose(b, 2 * (NW - 1))
                w_transpose(b, 2 * (NW - 1) + 1)

            def sv_chunks(b, t0, t1):
                """pure DR S@V over l-pair chunks [t0, t1)."""
                if t0 == 0:
                    ops_b[b] = [pp.tile([P, 512], f32, tag=f"o{2 * b + j}",
                                        name=f"sv{b}{j}") for j in range(2)]
                ops = ops_b[b]
                for t_ in range(t0, t1):
                    vt = vts[(b, t_ // 4)]
                    tt = t_ % 4
                    for j in range(2):
                        nc.tensor.matmul(
                            ops[j][:], wts[b][t_][:],
                            vt[:, 2 * tt:2 * tt + 2, j * 512:(j + 1) * 512],
                            start=(t_ == 0), stop=False, perf_mode=DR,
                        )
                    del wts[b][t_]

            def scores_newkey_a(b):
                # new-key scores + normalization sums + wn_pad staging; the
                # DVE tail here runs under whatever PE work follows.
                spn = pp.tile([P, 512], f32, tag=f"s{b % 2}", name="spn")
                for ci in range(0, CH, 2):
                    nc.tensor.matmul(
                        spn[:, 0:T], Qb[b][:, ci:ci + 2, :],
                        kT[:, ci:ci + 2, b * T:(b + 1) * T],
                        start=(ci == 0), stop=(ci == CH - 2), perf_mode=DR,
                    )
                nc.scalar.activation(
                    W_s[b][:, L:LT], spn[:, 0:T], AF.Exp,
                    bias=eshift[:, 0:1], accum_out=sums[b][:, NW:NW + 1],
                )
                rs = big.tile([P, 1], f32, tag=f"rs{b}", name=f"rs{b}")
                nc.vector.tensor_reduce(out=rs[:], in_=sums[b][:],
                                        axis=AX.X, op=OP.add)
                rsum[b] = big.tile([P, 1], f32, tag=f"rsum{b}", name=f"rsum{b}")
                nc.vector.reciprocal(rsum[b][:], rs[:])
                nc.vector.tensor_copy(out=wn_pad[b][:, b * T:(b + 1) * T],
                                      in_=W_s[b][:, L:LT])

            def scores_newkey_b(b):
                tpn = pp.tile([P, P], f32, tag=f"t{b % 2}", name="tpn")
                nc.tensor.matmul(tpn[0:M, :], wn_pad[b][:], ident_8[:],
                                 start=True, stop=True)
                nc.vector.tensor_copy(out=wt32[b][0:M, :], in_=tpn[0:M, :])

            def sv_final(b):
                ops = ops_b[b]
                for j in range(2):
                    nc.tensor.matmul(
                        ops[j][:], wt32[b][:], vpad[:, j * 512:(j + 1) * 512],
                        start=False, stop=True,
                    )
                    nc.scalar.activation(
                        On[b][:, j * 512:(j + 1) * 512], ops[j][:], AF.Copy,
                        scale=rsum[b][:],
                    )

            wvT = big.tile([P, CH, M], fp8, tag="wvT", name="wvT")

            def gather_ci(b, ci):
                tp = pp.tile([P, P], f32, tag=f"t{ci % 2}", name="tpg")
                nc.tensor.matmul(tp[:], On[b][:, ci * P:(ci + 1) * P],
                                 ident_b[:], start=True, stop=True)
                nc.vector.tensor_copy(
                    out=wvT[0:64, ci, b * T:(b + 1) * T],
                    in_=tp[0:64, 16 * ci:16 * ci + 8])
                nc.vector.tensor_copy(
                    out=wvT[64:P, ci, b * T:(b + 1) * T],
                    in_=tp[64:P, 16 * ci + 8:16 * ci + 16])

            # ---- k/v projections ----
            # attention-grade (fp8 DR) k/v for the core's own 2 batches;
            # output-grade (bf16) key/value c-slices for ALL batches.
            k_bf = big.tile([M, C], bf16, tag="k_bf", name="k_bf")
            kT = big.tile([P, CH, M], fp8, tag="kT", name="kT")
            ksl = natpool.tile([B * T, P], f32, tag="nat", name="ksl")
            vsl = natpool.tile([B * T, P], f32, tag="nat", name="vsl")

            def kv_proj(half):
                for j in range(2):
                    ps = pp.tile([M, 512], f32, tag=f"o{2 + j}",
                                 name=f"kv{half}{j}")
                    for ci in range(0, CH, 2):
                        nc.tensor.matmul(
                            ps[:], xT8_sb[:, ci:ci + 2, :],
                            wkv_sb[:, half, ci:ci + 2, j * 512:(j + 1) * 512],
                            start=(ci == 0), stop=(ci == CH - 2), perf_mode=DR,
                        )
                    sl = slice(j * 512, (j + 1) * 512)
                    if half == 0:
                        nc.vector.tensor_copy(out=k_bf[:, sl], in_=ps[:])
                    else:
                        nc.vector.tensor_add(out=vpad[0:M, sl], in0=ps[:],
                                             in1=bvb_sb[:, sl])

            def slice_proj():
                # key/value output c-slice for ALL 16 batches (bf16 grade)
                ps = pp.tile([B * T, 2 * P], f32, tag="o2", name="psl")
                for ci in range(CH):
                    nc.tensor.matmul(
                        ps[:], xall_sb[:, ci, :], wsl_sb[:, ci, :],
                        start=(ci == 0), stop=(ci == CH - 1),
                    )
                nc.scalar.copy(ksl[:], ps[:, 0:P])
                nc.vector.tensor_add(out=vsl[:], in0=ps[:, P:2 * P],
                                     in1=bvsl_sb[:])
                nc.scalar.dma_start(ksl_d[:], ksl[:])
                nc.scalar.dma_start(vsl_d[:], vsl[:])

            def kT_piece():
                for ci in range(CH):
                    tp = pp.tile([P, P], f32, tag=f"t{ci % 2}", name="tpk")
                    nc.tensor.matmul(tp[:, 0:M], k_bf[:, ci * P:(ci + 1) * P],
                                     ident_b[0:M, 0:M], start=True, stop=True)
                    nc.vector.tensor_copy(out=kT[:, ci, :], in_=tp[:, 0:M])

            # ============ main schedule (consumption-order aligned) ========
            for w in range(NW):                      # scores(0) + T(0)
                scores_window(0, w)
            scores_tail(0)
            sv_chunks(0, 0, 4)                       # <- V0s0
            scores_window(1, 0)                      # <- K1w01
            scores_window(1, 1)
            kv_proj(0)                               # <- Wk
            kT_piece()
            scores_newkey_a(0)
            sv_chunks(0, 4, 8)                       # <- V0s1
            scores_newkey_b(0)
            scores_window(1, 2)                      # <- K1w23
            scores_window(1, 3)
            slice_proj()                             # <- Wsl/xTall
            sv_chunks(0, 8, 12)                      # <- V0s2
            scores_window(1, 4)                      # <- K1w45
            scores_window(1, 5)
            sv_chunks(0, 12, 16)                     # <- V0s3
            scores_window(1, 6)                      # <- K1w67
            scores_window(1, 7)
            scores_tail(1)
            scores_newkey_a(1)
            kv_proj(1)                               # <- Wv
            scores_newkey_b(1)
            sv_final(0)
            for ci in range(CH):
                gather_ci(0, ci)
            sv_chunks(1, 0, 4)                       # <- V1s0
            sv_chunks(1, 4, 8)
            sv_chunks(1, 8, 12)
            sv_chunks(1, 12, 16)
            sv_final(1)

            # gather(1) + out projection (DR), 1-pair chase     <- Wo
            ps_fin = [pp.tile([M, 512], f32, tag=f"s{j}", name=f"fin{j}")
                      for j in range(2)]
            fin = natpool.tile([M, C], f32, tag="nat", name="fin")
            gather_ci(1, 0)
            gather_ci(1, 1)
            for pair in range(CH // 2):
                if pair < 3:
                    gather_ci(1, 2 * pair + 2)
                    gather_ci(1, 2 * pair + 3)
                ci = 2 * pair
                for j in range(2):
                    nc.tensor.matmul(
                        ps_fin[j][:], wvT[:, ci:ci + 2, :],
                        wo_sb[:, ci:ci + 2, j * 512:(j + 1) * 512],
                        start=(pair == 0), stop=(pair == CH // 2 - 1),
                        perf_mode=DR,
                    )
            for j in range(2):
                sl = slice(j * 512, (j + 1) * 512)
                nc.vector.tensor_add(out=fin[:, sl], in0=ps_fin[j][:],
                                     in1=bob_sb[:, sl])
                nc.scalar.dma_start(out_d[:, sl], fin[:, sl])

    nc.compile()
    return nc


def _prep_host(x, kv_cache, Wq, bq, Wk, Wv, bv, Wo, bo):
    fp8 = ml_dtypes.float8_e4m3
    bf16 = ml_dtypes.bfloat16
    f32 = np.float32
    x = np.asarray(x, f32)
    kv = np.asarray(kv_cache)
    Wq = np.asarray(Wq, f32); bq = np.asarray(bq, f32)
    Wk = np.asarray(Wk, f32); Wv = np.asarray(Wv, f32); bv = np.asarray(bv, f32)
    Wo = np.asarray(Wo, f32); bo = np.asarray(bo, f32)

    # K-cache / V-cache repacked so every device DMA is a fully contiguous
    # [128 x >=4KB] transfer:
    #   KT[b, w, p, ci*512 + j] = K[b, w*512 + j, ci*128 + p]
    #   Vd[b, s, p, tt*C + c]   = V[b, (s*NV + tt)*128 + p, c]
    KT_all = np.asarray(kv[:, 1, 0], f32).transpose(0, 2, 1).reshape(
        B, CH, P, NW, 512).transpose(0, 3, 2, 1, 4)
    # regroup into contiguous [P x 8KB] window-pair transfers:
    # KT2[b, wp, p, i*4096 + ci*512 + j] = window (2wp+i)
    KT_all = np.ascontiguousarray(KT_all).astype(fp8).reshape(
        B, NW // 2, 2, P, CH * 512).transpose(0, 1, 3, 2, 4)
    KT_all = np.ascontiguousarray(KT_all)
    V_all = np.asarray(kv[:, 1, 1], f32).reshape(
        B, L // (P * NV), NV, P, C).transpose(0, 1, 3, 2, 4)
    V_all = np.ascontiguousarray(V_all).astype(fp8)

    # weights: [P, CH, C(out)] with c_in = ci*128 + p
    Wq8 = np.ascontiguousarray(
        (Wq.T * SCALE).reshape(CH, P, C).transpose(1, 0, 2)).astype(fp8)
    Wo8 = np.ascontiguousarray(
        Wo.T.reshape(CH, P, C).transpose(1, 0, 2)).astype(fp8)
    Wkv8 = np.ascontiguousarray(np.stack([
        Wk.T.reshape(CH, P, C).transpose(1, 0, 2),
        Wv.T.reshape(CH, P, C).transpose(1, 0, 2)])).astype(fp8)
    bqs = np.ascontiguousarray((bq * SCALE).reshape(CH, P).T)  # [P, CH]
    bvb = np.ascontiguousarray(np.tile(bv, (M, 1)))
    bob = np.ascontiguousarray(np.tile(bo, (M, 1)))
    # all-batch x, transposed (for the key/value output slice projection)
    xall = x.reshape(B * T, C)
    xTall = np.ascontiguousarray(
        xall.reshape(B * T, CH, P).transpose(2, 1, 0)).astype(bf16)

    in_maps = []
    for c in range(NCORES):
        xc = x[c * BPC:(c + 1) * BPC].reshape(M, C)
        xT = np.ascontiguousarray(xc.reshape(M, CH, P).transpose(2, 1, 0))
        csl = slice(c * P, (c + 1) * P)
        Wslc = np.concatenate([Wk.T[:, csl], Wv.T[:, csl]], axis=1)
        Wslc = np.ascontiguousarray(
            Wslc.reshape(CH, P, 2 * P).transpose(1, 0, 2)).astype(bf16)
        in_maps.append({
            "xT8": xT.astype(fp8),
            "xTall": xTall,
            "Wq8": Wq8.reshape(P, CH * C),
            "Wo8": Wo8.reshape(P, CH * C),
            "Wkv8": Wkv8.reshape(2, P, CH * C),
            "Wsl": Wslc.reshape(P, CH * 2 * P),
            "bvsl": np.ascontiguousarray(np.tile(bv[csl], (B * T, 1))),
            "KT": np.ascontiguousarray(KT_all[c * BPC:(c + 1) * BPC]).reshape(
                BPC, NW // 2, P, 2 * CH * 512),
            "Vd": np.ascontiguousarray(V_all[c * BPC:(c + 1) * BPC]).reshape(
                BPC, L // (P * NV), P, NV * C),
            "bqs": bqs, "bvb": bvb, "bob": bob,
        })
    return in_maps


def kernel(x, kv_cache, Wq, bq, Wk, Wv, bv, Wo, bo, _trace=False, _tmpdir=None):
    from concourse.bass_utils import run_bass_kernel_spmd

    _ensure_ntff_hook()
    if "nc" not in _CACHE:
        _CACHE["nc"] = _build()
    nc = _CACHE["nc"]

    in_maps = _prep_host(x, kv_cache, Wq, bq, Wk, Wv, bv, Wo, bo)
    res = run_bass_kernel_spmd(
        nc, in_maps, core_ids=list(range(NCORES)),
        trace=_trace, tmpdir=_tmpdir,
    )
    out = np.empty((B, T, C), np.float32)
    key_o = np.empty((B, T, C), np.float32)
    val_o = np.empty((B, T, C), np.float32)
    for c in range(NCORES):
        r = res.results[c]
        sl = slice(c * BPC, (c + 1) * BPC)
        out[sl] = r["out"].reshape(BPC, T, C)
        csl = slice(c * P, (c + 1) * P)
        key_o[:, :, csl] = r["key_sl"].reshape(B, T, P)
        val_o[:, :, csl] = r["val_sl"].reshape(B, T, P)
    kernel._last_exec_time_ns = res.exec_time_ns
    kernel._last_results = res
    return (out, key_o, val_o)


# revision 44
# speedup vs baseline: 1.1311x; 1.0159x over previous
"""Trainium2 Bass kernel for cached multi-head self-attention decode step.

Problem (hardcoded):
  B=16, T=8, C=1024, n_head=16, head_dim=64, Lcache=4096, layer index 1.
  reference:
    q = x@Wq.T + bq ; key = x@Wk.T ; value = x@Wv.T + bv
    K = concat(kv_cache[:,1,0], key) ; V = concat(kv_cache[:,1,1], value)
    out = softmax((q*s)(K*s)^T) @ V @ Wo.T + bo      (s = hd**-0.25)
    returns (out, key, value)

Sharding: data-parallel over batch. 8 cores x 2 batches each. No collectives.

v4 design:
  - fp8 DoubleRow matmuls (contract 256/instr, measured ~225ns warm at
    N=512 - 2x bf16 per contract) for scores, S@V, q-proj, out-proj.
  - all transposes via matmul with identity rhs.
  - ONE ordered sync-queue DMA stream; PE program order aligned with DMA
    completion order so the PE never head-of-line blocks:
      x,Wq | KT0 | V0s0 | K1w01 | Wk | V0s1 | K1w23 | V0s2 | K1w45 |
      V0s3 | K1w67 | Wv | V1 | Wo
    PE: warmup, q, scores0+T0, sv0(c0-3), s1w01+T1, kproj+kT+newkey0,
      sv0(c4-7), s1w23, sv0(c8-11), s1w45, sv0(c12-15), s1w67, vproj,
      svfinal0, On0, newkey1, gather0, sv1, svfinal1, On1, gather1+outproj.
  - PE kept dense so the HAM clock gate stays at 8/8 (cold MMs are 2x).
"""

import sys
import types

import numpy as np
import ml_dtypes

# ---- hardcoded problem geometry ----
B, T, C = 16, 8, 1024
H, HD = 16, 64
L = 4096            # cached length
LT = L + T          # total keys
NCORES = 8
BPC = B // NCORES   # batches per core = 2
M = BPC * T         # queries per core = 16
P = 128
CH = C // P         # 8 c-chunks
NW = L // 512       # 8 score windows of 512
NV = 8              # V l-chunks (128 rows) per DMA tile (1MB transfers)
NDR = L // 256      # 16 DoubleRow l-pair chunks per batch
NWARM = 21          # PE warmup matmuls of N=512 (HAM un-throttle + stay busy
#                     until the first weights land ~15us in)
SCALE = float(HD) ** -0.5  # folded into Wq/bq on host

# softmax logit shift: exp(s + ESHIFT); cancels in normalization, keeps the
# fp8 S@V weights well inside e4m3 range.
ESHIFT = -2.0

_CACHE = {}


def _ensure_ntff_hook():
    """run_bass_kernel_spmd(trace=True) under axon needs antenv.axon_hooks;
    shim it from the boot module if the image's antenv lacks it."""
    try:
        import antenv.axon_hooks  # noqa: F401
        return
    except ImportError:
        pass
    try:
        import trn_agent_boot.trn_boot as tb
        hook = tb._ntff_profile_via_ctypes("/opt/axon/libaxon_pjrt.so")
    except Exception:
        hook = None
    mod = types.ModuleType("antenv.axon_hooks")
    mod.get_axon_ntff_profile_hook = lambda: hook
    mod.set_axon_ntff_profile_hook = lambda h: None
    sys.modules["antenv.axon_hooks"] = mod


def _build():
    import concourse.bacc as bacc
    import concourse.mybir as mybir
    import concourse.tile as tile
    from concourse.masks import make_identity

    f32 = mybir.dt.float32
    bf16 = mybir.dt.bfloat16
    fp8 = mybir.dt.float8e4
    DR = mybir.MatmulPerfMode.DoubleRow

    nc = bacc.Bacc(None, target_bir_lowering=False)

    # ---- dram I/O (all host-repacked for contiguous loads) ----
    xT8 = nc.dram_tensor("xT8", [P, CH, M], fp8, kind="ExternalInput")
    Wq8 = nc.dram_tensor("Wq8", [P, CH * C], fp8, kind="ExternalInput")
    Wo8 = nc.dram_tensor("Wo8", [P, CH * C], fp8, kind="ExternalInput")
    Wkv8 = nc.dram_tensor("Wkv8", [2, P, CH * C], fp8, kind="ExternalInput")
    xTall = nc.dram_tensor("xTall", [P, CH, B * T], bf16,
                           kind="ExternalInput")
    Wsl = nc.dram_tensor("Wsl", [P, CH * 2 * P], bf16, kind="ExternalInput")
    bvsl = nc.dram_tensor("bvsl", [B * T, P], f32, kind="ExternalInput")
    KT = nc.dram_tensor("KT", [BPC, NW // 2, P, 2 * CH * 512], fp8,
                        kind="ExternalInput")
    Vd = nc.dram_tensor("Vd", [BPC, L // (P * NV), P, NV * C], fp8,
                        kind="ExternalInput")
    bqs = nc.dram_tensor("bqs", [P, CH], f32, kind="ExternalInput")
    bvb = nc.dram_tensor("bvb", [M, C], f32, kind="ExternalInput")
    bob = nc.dram_tensor("bob", [M, C], f32, kind="ExternalInput")
    out_d = nc.dram_tensor("out", [M, C], f32, kind="ExternalOutput")
    ksl_d = nc.dram_tensor("key_sl", [B * T, P], f32, kind="ExternalOutput")
    vsl_d = nc.dram_tensor("val_sl", [B * T, P], f32, kind="ExternalOutput")

    AF = mybir.ActivationFunctionType
    AX = mybir.AxisListType
    OP = mybir.AluOpType

    with tile.TileContext(nc) as tc:
        with (
            tc.tile_pool(name="const", bufs=1) as cpool,
            tc.tile_pool(name="kt", bufs=8) as ktpool,
            tc.tile_pool(name="v", bufs=8) as vpool,
            tc.tile_pool(name="nat", bufs=2) as natpool,
            tc.tile_pool(name="wchunk", bufs=2 * NW + 2) as wtpool,
            tc.tile_pool(name="big", bufs=1) as big,
            tc.tile_pool(name="ps", bufs=1, space="PSUM") as pp,
        ):
            # ---------------- DMA issue order == consumption order --------
            xT8_sb = cpool.tile([P, CH, M], fp8, tag="xT8", name="xT8")
            nc.sync.dma_start(xT8_sb[:], xT8[:])
            wq_sb = cpool.tile([P, CH, C], fp8, tag="wq", name="wq")
            nc.sync.dma_start(wq_sb[:], Wq8[:])
            kts = {}

            def kt_dma(b, wp):
                # one contiguous 1MB transfer = two score windows (2w, 2w+1)
                kts[(b, wp)] = ktpool.tile([P, 2, CH, 512], fp8, tag="kt",
                                           name="kt")
                nc.sync.dma_start(kts[(b, wp)][:], KT[b, wp])

            vts = {}

            def v_dma(b, s):
                vts[(b, s)] = vpool.tile([P, NV, C], fp8, tag="v", name="v")
                nc.sync.dma_start(vts[(b, s)][:], Vd[b, s])

            wkv_sb = cpool.tile([P, 2, CH, C], fp8, tag="wkv", name="wkv")
            xall_sb = cpool.tile([P, CH, B * T], bf16, tag="xall", name="xall")
            wsl_sb = cpool.tile([P, CH, 2 * P], bf16, tag="wsl", name="wsl")
            bvsl_sb = cpool.tile([B * T, P], f32, tag="bvsl", name="bvsl")

            for wp in range(NW // 2):
                kt_dma(0, wp)
            v_dma(0, 0)
            kt_dma(1, 0)
            nc.sync.dma_start(wkv_sb[:, 0], Wkv8[0])   # Wk fp8
            v_dma(0, 1)
            kt_dma(1, 1)
            nc.sync.dma_start(xall_sb[:], xTall[:])
            nc.sync.dma_start(wsl_sb[:], Wsl[:])
            nc.sync.dma_start(bvsl_sb[:], bvsl[:])
            v_dma(0, 2)
            kt_dma(1, 2)
            v_dma(0, 3)
            kt_dma(1, 3)
            nc.sync.dma_start(wkv_sb[:, 1], Wkv8[1])   # Wv fp8
            for s in range(4):
                v_dma(1, s)
            wo_sb = cpool.tile([P, CH, C], fp8, tag="wo", name="wo")
            nc.sync.dma_start(wo_sb[:], Wo8[:])

            # ---- constants / memsets (gpsimd memsets BEFORE its slow
            # SWDGE dma emissions so the PE warmup input is ready early) ----
            warm_in = cpool.tile([P, 512], bf16, tag="warm_in", name="warm_in")
            nc.gpsimd.memset(warm_in[:], 0.25)
            eshift = cpool.tile([P, 1], f32, tag="eshift", name="eshift")
            nc.gpsimd.memset(eshift[:], ESHIFT)
            Qb = {}
            for b in range(BPC):
                Qb[b] = big.tile([P, CH, P], fp8, tag=f"Qbig{b}",
                                 name=f"Qbig{b}")
                nc.gpsimd.memset(Qb[b][:], 0.0)
            vpad = big.tile([P, C], fp8, tag="vpad", name="vpad")
            nc.gpsimd.memset(vpad[:], 0.0)
            wn_pad, wt32 = {}, {}
            for b in range(BPC):
                wn_pad[b] = big.tile([P, M], fp8, tag=f"wn_pad{b}",
                                     name=f"wn_pad{b}")
                nc.gpsimd.memset(wn_pad[b][:], 0.0)
                wt32[b] = big.tile([P, P], fp8, tag=f"wt32_{b}",
                                   name=f"wt32_{b}")
                nc.gpsimd.memset(wt32[b][:], 0.0)
            bqs_sb = cpool.tile([P, CH], f32, tag="bqs", name="bqs")
            nc.gpsimd.dma_start(bqs_sb[:], bqs[:])
            bvb_sb = cpool.tile([M, C], bf16, tag="bvb", name="bvb")
            nc.gpsimd.dma_start(bvb_sb[:], bvb[:])
            bob_sb = cpool.tile([M, C], bf16, tag="bob", name="bob")
            nc.gpsimd.dma_start(bob_sb[:], bob[:])

            ident = cpool.tile([P, P], f32, tag="ident", name="ident")
            make_identity(nc, ident)
            ident_b = cpool.tile([P, P], bf16, tag="ident_b", name="ident_b")
            nc.vector.tensor_copy(out=ident_b[:], in_=ident[:])
            ident_8 = cpool.tile([P, P], fp8, tag="ident_8", name="ident_8")
            nc.vector.tensor_copy(out=ident_8[:], in_=ident[:])

            # ---- PE warmup / filler: keep the HAM clock gate at 8/8 (cold
            # matmuls run at 1.2 vs 2.4 GHz). Each call allocates from the
            # t-tag ring so WAR deps are tracked.
            _wk = [0]

            def warm_fill(n, nfree=64):
                wm = pp.tile([P, 512], f32, tag=f"t{_wk[0] % 2}", name="warm")
                _wk[0] += 1
                for _ in range(n):
                    nc.tensor.matmul(wm[:, 0:nfree], warm_in[:, 0:P],
                                     warm_in[:, 0:nfree],
                                     start=True, stop=True)

            warm_fill(NWARM, nfree=512)

            # ---------------- per-batch state ------------------------------
            W_s, sums, rsum, ops_b, On = {}, {}, {}, {}, {}
            for b in range(BPC):
                W_s[b] = big.tile([P, LT], fp8, tag=f"W{b}", name=f"W{b}")
                sums[b] = big.tile([P, NW + 1], f32, tag=f"sums{b}",
                                   name=f"sums{b}")
                On[b] = big.tile([P, C], bf16, tag=f"On{b}", name=f"On{b}")
            wts = {0: {}, 1: {}}  # (b -> chunk -> wt tile)

            # ---------------- Phase A: q projection (fp8 DR) --------------
            q_bf = big.tile([M, C], bf16, tag="q_bf", name="q_bf")
            for j in range(2):
                qps = pp.tile([M, 512], f32, tag=f"s{j}", name=f"qps{j}")
                for ci in range(0, CH, 2):
                    nc.tensor.matmul(
                        qps[:], xT8_sb[:, ci:ci + 2, :],
                        wq_sb[:, ci:ci + 2, j * 512:(j + 1) * 512],
                        start=(ci == 0), stop=(ci == CH - 2), perf_mode=DR,
                    )
                nc.scalar.copy(q_bf[:, j * 512:(j + 1) * 512], qps[:])
            for co in range(CH):
                tpq = pp.tile([P, P], f32, tag=f"t{co % 2}", name="tpq")
                nc.tensor.matmul(
                    tpq[:, 0:M], q_bf[:, co * P:(co + 1) * P],
                    ident_b[0:M, 0:M], start=True, stop=True,
                )
                for b in range(BPC):
                    for j in range(2):
                        rows = slice(64 * j, 64 * (j + 1))
                        nc.scalar.activation(
                            Qb[b][rows, co, 16 * co + 8 * j:16 * co + 8 * j + 8],
                            tpq[rows, b * T:b * T + T],
                            AF.Identity, bias=bqs_sb[rows, co:co + 1],
                        )

            def w_transpose(b, t):
                """wt pair tile for DR l-chunk t of batch b (2 matmul-T)."""
                wt = wtpool.tile([P, 2, P], fp8, tag="wt", name="wt")
                for i in range(2):
                    tpw = pp.tile([P, P], f32, tag=f"t{(2 * t + i) % 2}",
                                  name="tpw")
                    nc.tensor.matmul(
                        tpw[:], W_s[b][:, (2 * t + i) * P:(2 * t + i + 1) * P],
                        ident_8[:], start=True, stop=True,
                    )
                    nc.vector.tensor_copy(out=wt[:, i, :], in_=tpw[:])
                wts[b][t] = wt

            def scores_window(b, lw, transpose=True):
                kt = kts[(b, lw // 2)][:, lw % 2]
                sp = pp.tile([P, 512], f32, tag=f"s{lw % 2}", name="sp")
                for ci in range(0, CH, 2):
                    nc.tensor.matmul(
                        sp[:], Qb[b][:, ci:ci + 2, :], kt[:, ci:ci + 2, :],
                        start=(ci == 0), stop=(ci == CH - 2), perf_mode=DR,
                    )
                nc.scalar.activation(
                    W_s[b][:, lw * 512:(lw + 1) * 512], sp[:], AF.Exp,
                    bias=eshift[:, 0:1],
                    accum_out=sums[b][:, lw:lw + 1],
                )
                if transpose and lw > 0:
                    w_transpose(b, 2 * (lw - 1))
                    w_transpose(b, 2 * (lw - 1) + 1)

            def scores_tail(b):
                w_transpose(b, 2 * (NW - 1))
                w_transpose(b, 2 * (NW - 1) + 1)

            def sv_chunks(b, t0, t1):
                """pure DR S@V over l-pair chunks [t0, t1)."""
                if t0 == 0:
                    ops_b[b] = [pp.tile([P, 512], f32, tag=f"o{2 * b + j}",
                                        name=f"sv{b}{j}") for j in range(2)]
                ops = ops_b[b]
                for t_ in range(t0, t1):
                    vt = vts[(b, t_ // 4)]
                    tt = t_ % 4
                    for j in range(2):
                        nc.tensor.matmul(
                            ops[j][:], wts[b][t_][:],
                            vt[:, 2 * tt:2 * tt + 2, j * 512:(j + 1) * 512],
                            start=(t_ == 0), stop=False, perf_mode=DR,
                        )
                    del wts[b][t_]

            def scores_newkey_a(b):
                # new-key scores + normalization sums + wn_pad staging; the
                # DVE tail here runs under whatever PE work follows.
                spn = pp.tile([P, 512], f32, tag=f"s{b % 2}", name="spn")
                for ci in range(0, CH, 2):
                    nc.tensor.matmul(
                        spn[:, 0:T], Qb[b][:, ci:ci + 2, :],
                        kT[:, ci:ci + 2, b * T:(b + 1) * T],
                        start=(ci == 0), stop=(ci == CH - 2), perf_mode=DR,
                    )
                nc.scalar.activation(
                    W_s[b][:, L:LT], spn[:, 0:T], AF.Exp,
                    bias=eshift[:, 0:1], accum_out=sums[b][:, NW:NW + 1],
                )
                rs = big.tile([P, 1], f32, tag=f"rs{b}", name=f"rs{b}")
                nc.vector.tensor_reduce(out=rs[:], in_=sums[b][:],
                                        axis=AX.X, op=OP.add)
                rsum[b] = big.tile([P, 1], f32, tag=f"rsum{b}", name=f"rsum{b}")
                nc.vector.reciprocal(rsum[b][:], rs[:])
                nc.vector.tensor_copy(out=wn_pad[b][:, b * T:(b + 1) * T],
                                      in_=W_s[b][:, L:LT])

            def scores_newkey_b(b):
                tpn = pp.tile([P, P], f32, tag=f"t{b % 2}", name="tpn")
                nc.tensor.matmul(tpn[0:M, :], wn_pad[b][:], ident_8[:],
                                 start=True, stop=True)
                nc.vector.tensor_copy(out=wt32[b][0:M, :], in_=tpn[0:M, :])

            def sv_final(b):
                ops = ops_b[b]
                for j in range(2):
                    nc.tensor.matmul(
                        ops[j][:], wt32[b][:], vpad[:, j * 512:(j + 1) * 512],
                        start=False, stop=True,
                    )
                    nc.scalar.activation(
                        On[b][:, j * 512:(j + 1) * 512], ops[j][:], AF.Copy,
                        scale=rsum[b][:],
                    )

            wvT = big.tile([P, CH, M], fp8, tag="wvT", name="wvT")

            def gather_ci(b, ci):
                tp = pp.tile([P, P], f32, tag=f"t{ci % 2}", name="tpg")
                nc.tensor.matmul(tp[:], On[b][:, ci * P:(ci + 1) * P],
                                 ident_b[:], start=True, stop=True)
                nc.vector.tensor_copy(
                    out=wvT[0:64, ci, b * T:(b + 1) * T],
                    in_=tp[0:64, 16 * ci:16 * ci + 8])
                nc.vector.tensor_copy(
                    out=wvT[64:P, ci, b * T:(b + 1) * T],
                    in_=tp[64:P, 16 * ci + 8:16 * ci + 16])

            # ---- k/v projections ----
            # attention-grade (fp8 DR) k/v for the core's own 2 batches;
            # output-grade (bf16) key/value c-slices for ALL batches.
            k_bf = big.tile([M, C], bf16, tag="k_bf", name="k_bf")
            kT = big.tile([P, CH, M], fp8, tag="kT", name="kT")
            ksl = natpool.tile([B * T, P], f32, tag="nat", name="ksl")
            vsl = natpool.tile([B * T, P], f32, tag="nat", name="vsl")

            def kv_proj(half):
                for j in range(2):
                    ps = pp.tile([M, 512], f32, tag=f"o{2 + j}",
                                 name=f"kv{half}{j}")
                    for ci in range(0, CH, 2):
                        nc.tensor.matmul(
                            ps[:], xT8_sb[:, ci:ci + 2, :],
                            wkv_sb[:, half, ci:ci + 2, j * 512:(j + 1) * 512],
                            start=(ci == 0), stop=(ci == CH - 2), perf_mode=DR,
                        )
                    sl = slice(j * 512, (j + 1) * 512)
                    if half == 0:
                        nc.vector.tensor_copy(out=k_bf[:, sl], in_=ps[:])
                    else:
                        nc.vector.tensor_add(out=vpad[0:M, sl], in0=ps[:],
                                             in1=bvb_sb[:, sl])

            def slice_proj():
                # key/value output c-slice for ALL 16 batches (bf16 grade)
                ps = pp.tile([B * T, 2 * P], f32, tag="o2", name="psl")
                for ci in range(CH):
                    nc.tensor.matmul(
                        ps[:], xall_sb[:, ci, :], wsl_sb[:, ci, :],
                        start=(ci == 0), stop=(ci == CH - 1),
                    )
                nc.scalar.copy(ksl[:], ps[:, 0:P])
                nc.vector.tensor_add(out=vsl[:], in0=ps[:, P:2 * P],
                                     in1=bvsl_sb[:])
                nc.scalar.dma_start(ksl_d[:], ksl[:])
                nc.scalar.dma_start(vsl_d[:], vsl[:])

            def kT_piece():
                for ci in range(CH):
                    tp = pp.tile([P, P], f32, tag=f"t{ci % 2}", name="tpk")
                    nc.tensor.matmul(tp[:, 0:M], k_bf[:, ci * P:(ci + 1) * P],
                                     ident_b[0:M, 0:M], start=True, stop=True)
                    nc.vector.tensor_copy(out=kT[:, ci, :], in_=tp[:, 0:M])

            # ============ main schedule (consumption-order aligned) ========
            for w in range(NW):                      # scores(0) + T(0)
                scores_window(0, w)
            scores_tail(0)
            sv_chunks(0, 0, 4)                       # <- V0s0
            scores_window(1, 0)                      # <- K1w01
            scores_window(1, 1)
            kv_proj(0)                               # <- Wk
            kT_piece()
            scores_newkey_a(0)
            sv_chunks(0, 4, 8)                       # <- V0s1
            scores_newkey_b(0)
            scores_window(1, 2)                      # <- K1w23
            scores_window(1, 3)
            slice_proj()                             # <- Wsl/xTall
            sv_chunks(0, 8, 12)                      # <- V0s2
            scores_window(1, 4)                      # <- K1w45
            scores_window(1, 5)
            sv_chunks(0, 12, 16)                     # <- V0s3
            scores_window(1, 6)                      # <- K1w67
            scores_window(1, 7)
            scores_tail(1)
            scores_newkey_a(1)
            kv_proj(1)                               # <- Wv
            scores_newkey_b(1)
            sv_final(0)
            for ci in range(CH):
                gather_ci(0, ci)
            sv_chunks(1, 0, 4)                       # <- V1s0
            sv_chunks(1, 4, 8)
            sv_chunks(1, 8, 12)
            sv_chunks(1, 12, 16)
            sv_final(1)

            # gather(1) + out projection (DR), 1-pair chase     <- Wo
            ps_fin = [pp.tile([M, 512], f32, tag=f"s{j}", name=f"fin{j}")
                      for j in range(2)]
            fin = natpool.tile([M, C], f32, tag="nat", name="fin")
            gather_ci(1, 0)
            gather_ci(1, 1)
            for pair in range(CH // 2):
                if pair < 3:
                    gather_ci(1, 2 * pair + 2)
                    gather_ci(1, 2 * pair + 3)
                ci = 2 * pair
                for j in range(2):
                    nc.tensor.matmul(
                        ps_fin[j][:], wvT[:, ci:ci + 2, :],
                        wo_sb[:, ci:ci + 2, j * 512:(j + 1) * 512],
                        start=(pair == 0), stop=(pair == CH // 2 - 1),
                        perf_mode=DR,
                    )
            for j in range(2):
                sl = slice(j * 512, (j + 1) * 512)
                nc.vector.tensor_add(out=fin[:, sl], in0=ps_fin[j][:],
                                     in1=bob_sb[:, sl])
                nc.scalar.dma_start(out_d[:, sl], fin[:, sl])

    nc.compile()
    return nc


def _prep_host(x, kv_cache, Wq, bq, Wk, Wv, bv, Wo, bo):
    fp8 = ml_dtypes.float8_e4m3
    bf16 = ml_dtypes.bfloat16
    f32 = np.float32
    x = np.asarray(x, f32)
    kv = np.asarray(kv_cache)
    Wq = np.asarray(Wq, f32); bq = np.asarray(bq, f32)
    Wk = np.asarray(Wk, f32); Wv = np.asarray(Wv, f32); bv = np.asarray(bv, f32)
    Wo = np.asarray(Wo, f32); bo = np.asarray(bo, f32)

    # K-cache / V-cache repacked so every device DMA is a fully contiguous
    # [128 x >=4KB] transfer:
    #   KT[b, w, p, ci*512 + j] = K[b, w*512 + j, ci*128 + p]
    #   Vd[b, s, p, tt*C + c]   = V[b, (s*NV + tt)*128 + p, c]
    KT_all = np.asarray(kv[:, 1, 0], f32).transpose(0, 2, 1).reshape(
        B, CH, P, NW, 512).transpose(0, 3, 2, 1, 4)
    # regroup into contiguous [P x 8KB] window-pair transfers:
    # KT2[b, wp, p, i*4096 + ci*512 + j] = window (2wp+i)
    KT_all = np.ascontiguousarray(KT_all).astype(fp8).reshape(
        B, NW // 2, 2, P, CH * 512).transpose(0, 1, 3, 2, 4)
    KT_all = np.ascontiguousarray(KT_all)
    V_all = np.asarray(kv[:, 1, 1], f32).reshape(
        B, L // (P * NV), NV, P, C).transpose(0, 1, 3, 2, 4)
    V_all = np.ascontiguousarray(V_all).astype(fp8)

    # weights: [P, CH, C(out)] with c_in = ci*128 + p
    Wq8 = np.ascontiguousarray(
        (Wq.T * SCALE).reshape(CH, P, C).transpose(1, 0, 2)).astype(fp8)
    Wo8 = np.ascontiguousarray(
        Wo.T.reshape(CH, P, C).transpose(1, 0, 2)).astype(fp8)
    Wkv8 = np.ascontiguousarray(np.stack([
        Wk.T.reshape(CH, P, C).transpose(1, 0, 2),
        Wv.T.reshape(CH, P, C).transpose(1, 0, 2)])).astype(fp8)
    bqs = np.ascontiguousarray((bq * SCALE).reshape(CH, P).T)  # [P, CH]
    bvb = np.ascontiguousarray(np.tile(bv, (M, 1)))
    bob = np.ascontiguousarray(np.tile(bo, (M, 1)))
    # all-batch x, transposed (for the key/value output slice projection)
    xall = x.reshape(B * T, C)
    xTall = np.ascontiguousarray(
        xall.reshape(B * T, CH, P).transpose(2, 1, 0)).astype(bf16)

    in_maps = []
    for c in range(NCORES):
        xc = x[c * BPC:(c + 1) * BPC].reshape(M, C)
        xT = np.ascontiguousarray(xc.reshape(M, CH, P).transpose(2, 1, 0))
        csl = slice(c * P, (c + 1) * P)
        Wslc = np.concatenate([Wk.T[:, csl], Wv.T[:, csl]], axis=1)
        Wslc = np.ascontiguousarray(
            Wslc.reshape(CH, P, 2 * P).transpose(1, 0, 2)).astype(bf16)
        in_maps.append({
            "xT8": xT.astype(fp8),
            "xTall": xTall,
            "Wq8": Wq8.reshape(P, CH * C),
            "Wo8": Wo8.reshape(P, CH * C),
            "Wkv8": Wkv8.reshape(2, P, CH * C),
            "Wsl": Wslc.reshape(P, CH * 2 * P),
            "bvsl": np.ascontiguousarray(np.tile(bv[csl], (B * T, 1))),
            "KT": np.ascontiguousarray(KT_all[c * BPC:(c + 1) * BPC]).reshape(
                BPC, NW // 2, P, 2 * CH * 512),
            "Vd": np.ascontiguousarray(V_all[c * BPC:(c + 1) * BPC]).reshape(
                BPC, L // (P * NV), P, NV * C),
            "bqs": bqs, "bvb": bvb, "bob": bob,
        })
    return in_maps


def kernel(x, kv_cache, Wq, bq, Wk, Wv, bv, Wo, bo, _trace=False, _tmpdir=None):
    from concourse.bass_utils import run_bass_kernel_spmd

    _ensure_ntff_hook()
    if "nc" not in _CACHE:
        _CACHE["nc"] = _build()
    nc = _CACHE["nc"]

    in_maps = _prep_host(x, kv_cache, Wq, bq, Wk, Wv, bv, Wo, bo)
    res = run_bass_kernel_spmd(
        nc, in_maps, core_ids=list(range(NCORES)),
        trace=_trace, tmpdir=_tmpdir,
    )
    out = np.empty((B, T, C), np.float32)
    key_o = np.empty((B, T, C), np.float32)
    val_o = np.empty((B, T, C), np.float32)
    for c in range(NCORES):
        r = res.results[c]
        sl = slice(c * BPC, (c + 1) * BPC)
        out[sl] = r["out"].reshape(BPC, T, C)
        csl = slice(c * P, (c + 1) * P)
        key_o[:, :, csl] = r["key_sl"].reshape(B, T, P)
        val_o[:, :, csl] = r["val_sl"].reshape(B, T, P)
    kernel._last_exec_time_ns = res.exec_time_ns
    kernel._last_results = res
    return (out, key_o, val_o)
